# revision 4
# baseline (speedup 1.0000x reference)
"""Trainium2 Bass kernel for nn_MoEResBlock — fused single-launch version.

Per core (8192 tokens): router -> top-2 + gates -> hierarchical cumsum
positions -> SWDGE scatter into per-(core,expert) DRAM regions -> on-device
AllGather of per-core expert counts (overlapped with the expert MLP) ->
dense per-expert MLP (fp16 matmuls, PE transposes, LN via augmented mean
column) -> SWDGE gather-combine with exact global-capacity gates ->
residual + relu -> bf16 out (host upconverts).
"""

import sys

for _p in ("/opt/trn_rl_repo",):
    if _p not in sys.path:
        sys.path.insert(0, _p)

from contextlib import ExitStack

import numpy as np

import concourse.bass as bass
import concourse.mybir as mybir
import concourse.tile as tile
from concourse import bacc
from concourse.bass_utils import run_bass_kernel_spmd
from concourse.masks import make_identity

F32 = mybir.dt.float32
I16 = mybir.dt.int16
I32 = mybir.dt.int32
BF16 = mybir.dt.float16  # half dtype for matmul operands (fp16: 10-bit mantissa)
AX = mybir.AxisListType
OP = mybir.AluOpType
ACTF = mybir.ActivationFunctionType

P = 128
D = 256
E = 8
NCORES = 8
TOK = 65536 // NCORES
NT = TOK // P
GRP = 8
MAXC = 2560
ETILES = MAXC // P
WV = 4
TRASH = E * MAXC
XROWS = TRASH + P
CAP = 16384
BIG = 1000.0
NEG = -1.0e30
LN_EPS = 1e-6


def build_fused(ln_ident=True):
    nc = bacc.Bacc("TRN2", target_bir_lowering=False, debug=False)

    x = nc.dram_tensor("x", [TOK, D], F32, kind="ExternalInput")
    wr = nc.dram_tensor("wr", [D, E], F32, kind="ExternalInput")
    br = nc.dram_tensor("br", [E], F32, kind="ExternalInput")
    w1d = nc.dram_tensor("w1d", [E, D, D], F32, kind="ExternalInput")
    b1d = nc.dram_tensor("b1d", [E, D], F32, kind="ExternalInput")
    s1d = nc.dram_tensor("s1d", [E, D], F32, kind="ExternalInput")
    c1d = nc.dram_tensor("c1d", [E, D], F32, kind="ExternalInput")
    w2d = nc.dram_tensor("w2d", [E, D, D], F32, kind="ExternalInput")
    b2d = nc.dram_tensor("b2d", [E, D], F32, kind="ExternalInput")
    s2d = nc.dram_tensor("s2d", [E, D], F32, kind="ExternalInput")
    c2d = nc.dram_tensor("c2d", [E, D], F32, kind="ExternalInput")
    mask_lt = nc.dram_tensor("mask_lt", [NCORES, 1], F32, kind="ExternalInput")

    out_o = nc.dram_tensor("out", [TOK, D], BF16, kind="ExternalOutput")
    # scatter-add target: ExternalOutput => guaranteed zero-initialized
    xin_bf = nc.dram_tensor("xin", [XROWS, D], BF16, kind="ExternalOutput")
    y_all = nc.dram_tensor("y_all", [XROWS, D], BF16)

    with tile.TileContext(nc) as tc, ExitStack() as ctx:
        consts = ctx.enter_context(tc.tile_pool(name="consts", bufs=1))
        bigp = ctx.enter_context(tc.tile_pool(name="bigp", bufs=1))
        dram = ctx.enter_context(tc.tile_pool(name="dram", bufs=2, space="DRAM"))

        ident = consts.tile([P, P], F32)
        make_identity(nc, ident[:])
        ident16 = consts.tile([P, P], BF16)
        nc.vector.tensor_copy(ident16[:], ident[:])
        # SL[p, i] = 1.0 iff p < i
        sl_ci = consts.tile([P, P], I32)
        nc.gpsimd.iota(sl_ci[:], pattern=[[1, P]], base=0, channel_multiplier=0)
        sl_ri = consts.tile([P, P], I32)
        nc.gpsimd.iota(sl_ri[:], pattern=[[0, P]], base=0, channel_multiplier=1)
        sl_c = consts.tile([P, P], F32)
        nc.vector.tensor_copy(sl_c[:], sl_ci[:])
        sl_r = consts.tile([P, P], F32)
        nc.vector.tensor_copy(sl_r[:], sl_ri[:])
        sl = consts.tile([P, P], F32)
        nc.vector.tensor_tensor(out=sl[:], in0=sl_r[:], in1=sl_c[:], op=OP.is_lt)
        iota_i = consts.tile([P, E], I32)
        nc.gpsimd.iota(iota_i[:], pattern=[[1, E]], base=0, channel_multiplier=0)
        iota_f = consts.tile([P, E], F32)
        nc.vector.tensor_copy(iota_f[:], iota_i[:])
        iota_mb = consts.tile([P, E], F32)
        nc.vector.tensor_scalar_add(iota_mb[:], iota_i[:], -BIG)
        ones_col = consts.tile([P, 1], F32)
        nc.vector.memset(ones_col[:], 1.0)
        eps_t = consts.tile([P, 1], F32)
        nc.vector.memset(eps_t[:], LN_EPS)

        br_row = consts.tile([1, E], F32)
        nc.sync.dma_start(br_row[:], br[None, :])
        br_bc = consts.tile([P, E], F32)
        nc.gpsimd.partition_broadcast(br_bc[:], br_row[:])
        wr_sb = consts.tile([P, 2, E], F32)
        nc.sync.dma_start(wr_sb[:], wr.rearrange("(k p) e -> p k e", p=P))
        mlt_sb = consts.tile([NCORES, 1], F32)
        nc.sync.dma_start(mlt_sb[:], mask_lt[:])

        # ---- bulk x load (single read, reused by router/scatter/combine) ----
        x_all = bigp.tile([P, NT, D], F32)
        nc.sync.dma_start(x_all[:], x.rearrange("(t p) d -> p t d", p=P))

        s_all = bigp.tile([P, NT, E], F32)
        m1_all = bigp.tile([P, NT, E], F32)
        m2_all = bigp.tile([P, NT, E], F32)
        idx1_sb = bigp.tile([P, NT], F32)
        idx2_sb = bigp.tile([P, NT], F32)
        g1_sb = bigp.tile([P, NT], F32)
        g2_sb = bigp.tile([P, NT], F32)

        # =================== router ===================
        with tc.tile_pool(name="rxp", bufs=3) as xp, \
             tc.tile_pool(name="rtp", bufs=3) as tp, \
             tc.tile_pool(name="rsm", bufs=4) as sm, \
             tc.tile_pool(name="rps", bufs=2, space="PSUM") as ps, \
             tc.tile_pool(name="rpl", bufs=1, space="PSUM") as pl:

            for g in range(NT // GRP):
                lg = tp.tile([P, GRP, E], F32)
                for t in range(GRP):
                    ti = g * GRP + t
                    xts = tp.tile([P, 2, P], F32, tag="xts")
                    for k in range(2):
                        xt_ps = ps.tile([P, P], F32)
                        nc.tensor.transpose(xt_ps[:], x_all[:, ti, k * P:(k + 1) * P],
                                            ident[:])
                        nc.scalar.copy(xts[:, k, :], xt_ps[:])
                    lg_ps = ps.tile([P, E], F32, tag="lgps")
                    for k in range(2):
                        nc.tensor.matmul(lg_ps[:], lhsT=xts[:, k, :],
                                         rhs=wr_sb[:, k, :],
                                         start=(k == 0), stop=(k == 1))
                    nc.vector.tensor_add(lg[:, t, :], lg_ps[:], br_bc[:])

                gb = slice(g * GRP, (g + 1) * GRP)
                iota_b = iota_mb[:, None, :].to_broadcast([P, GRP, E])
                m1 = sm.tile([P, GRP, 1], F32)
                nc.vector.tensor_reduce(m1[:], lg[:], axis=AX.X, op=OP.max)
                eq1 = tp.tile([P, GRP, E], F32, tag="eq")
                nc.vector.tensor_tensor(out=eq1[:], in0=lg[:],
                                        in1=m1[:].to_broadcast([P, GRP, E]),
                                        op=OP.is_equal)
                cand = tp.tile([P, GRP, E], F32, tag="cand")
                nc.vector.tensor_tensor(out=cand[:], in0=eq1[:], in1=iota_b,
                                        op=OP.mult)
                i1m = sm.tile([P, GRP, 1], F32)
                nc.vector.tensor_reduce(i1m[:], cand[:], axis=AX.X, op=OP.min)
                nc.vector.tensor_scalar_add(idx1_sb[:, gb], i1m[:, :, 0], BIG)
                nc.vector.tensor_tensor(out=m1_all[:, gb, :], in0=iota_b,
                                        in1=i1m[:].to_broadcast([P, GRP, E]),
                                        op=OP.is_equal)
                l2 = tp.tile([P, GRP, E], F32, tag="l2")
                nc.vector.scalar_tensor_tensor(out=l2[:], in0=m1_all[:, gb, :],
                                               scalar=NEG, in1=lg[:],
                                               op0=OP.mult, op1=OP.add)
                m2 = sm.tile([P, GRP, 1], F32)
                nc.vector.tensor_reduce(m2[:], l2[:], axis=AX.X, op=OP.max)
                eq2 = tp.tile([P, GRP, E], F32, tag="eq")
                nc.vector.tensor_tensor(out=eq2[:], in0=l2[:],
                                        in1=m2[:].to_broadcast([P, GRP, E]),
                                        op=OP.is_equal)
                cand2 = tp.tile([P, GRP, E], F32, tag="cand")
                nc.vector.tensor_tensor(out=cand2[:], in0=eq2[:], in1=iota_b,
                                        op=OP.mult)
                i2m = sm.tile([P, GRP, 1], F32)
                nc.vector.tensor_reduce(i2m[:], cand2[:], axis=AX.X, op=OP.min)
                nc.vector.tensor_scalar_add(idx2_sb[:, gb], i2m[:, :, 0], BIG)
                nc.vector.tensor_tensor(out=m2_all[:, gb, :], in0=iota_b,
                                        in1=i2m[:].to_broadcast([P, GRP, E]),
                                        op=OP.is_equal)
                nc.vector.tensor_tensor(out=s_all[:, gb, :], in0=m1_all[:, gb, :],
                                        in1=m2_all[:, gb, :], op=OP.add)
                dsc = sm.tile([P, GRP, 1], F32)
                nc.vector.tensor_tensor(out=dsc[:], in0=m2[:], in1=m1[:],
                                        op=OP.subtract)
                edv = sm.tile([P, GRP, 1], F32)
                nc.scalar.activation(edv[:], dsc[:], ACTF.Exp)
                nc.vector.tensor_scalar_add(edv[:], edv[:], 1.0)
                g1t = sm.tile([P, GRP, 1], F32)
                nc.vector.reciprocal(g1t[:], edv[:])
                nc.vector.tensor_copy(g1_sb[:, gb], g1t[:, :, 0])
                nc.vector.tensor_scalar(out=g2_sb[:, gb], in0=g1t[:, :, 0],
                                        scalar1=-1.0, scalar2=1.0,
                                        op0=OP.mult, op1=OP.add)

            # ------- hierarchical exclusive cumsum over pair order -------
            s_flat = s_all[:].rearrange("p t e -> p (t e)")
            cab_ps = pl.tile([P, NT * E], F32)
            nc.tensor.matmul(cab_ps[:], lhsT=sl[:], rhs=s_flat, start=True, stop=True)
            cab_sb = bigp.tile([P, NT, E], F32)
            nc.scalar.copy(cab_sb[:].rearrange("p t e -> p (t e)"), cab_ps[:])

            trow_ps = pl.tile([1, NT * E], F32, tag="trow")
            nc.tensor.matmul(trow_ps[:], lhsT=ones_col[:], rhs=s_flat,
                             start=True, stop=True)
            trow_sb = sm.tile([1, NT * E], F32, tag="trowsb")
            nc.scalar.copy(trow_sb[:], trow_ps[:])
            t_p = sm.tile([NT, E], F32, tag="tp64")
            nc.sync.dma_start(t_p[:], trow_sb[:])
            toff_ps = pl.tile([NT, E], F32, tag="toffps")
            nc.tensor.matmul(toff_ps[:], lhsT=sl[:NT, :NT], rhs=t_p[:],
                             start=True, stop=True)
            toff_sb = sm.tile([NT, E], F32, tag="toffsb")
            nc.scalar.copy(toff_sb[:], toff_ps[:])
            toff_row = sm.tile([1, NT * E], F32, tag="toffrow")
            nc.sync.dma_start(toff_row[:], toff_sb[:])
            toff_bc = bigp.tile([P, NT, E], F32)
            nc.gpsimd.partition_broadcast(toff_bc[:].rearrange("p t e -> p (t e)"),
                                          toff_row[:])
            nc.vector.tensor_add(cab_sb[:], cab_sb[:], toff_bc[:])

            cnt_ps = pl.tile([1, E], F32, tag="cntps")
            nc.tensor.matmul(cnt_ps[:], lhsT=ones_col[:NT, :], rhs=t_p[:],
                             start=True, stop=True)
            cnt_sb = sm.tile([1, E], F32, tag="cntsb")
            nc.scalar.copy(cnt_sb[:], cnt_ps[:])
            # collective input bounce
            cin_b = dram.tile([1, E], F32)
            nc.sync.dma_start(cin_b[:], cnt_sb[:])

            # ------- per-pair local positions + dispatch locations -------
            tmp = bigp.tile([P, NT, E], F32)
            lpos = [None, None]
            for s_i, mask in ((0, m1_all), (1, m2_all)):
                nc.vector.tensor_tensor(out=tmp[:], in0=mask[:], in1=cab_sb[:],
                                        op=OP.mult)
                lp = bigp.tile([P, NT], F32, tag=f"lpos{s_i}")
                nc.vector.tensor_reduce(lp[:], tmp[:], axis=AX.X, op=OP.add)
                lpos[s_i] = lp

            trash_t = consts.tile([P, NT], F32)
            nc.vector.memset(trash_t[:], float(TRASH))
            loc_i16 = []
            for s_i, idxs in ((0, idx1_sb), (1, idx2_sb)):
                loc = bigp.tile([P, NT], F32, tag=f"loc{s_i}")
                nc.vector.scalar_tensor_tensor(out=loc[:], in0=idxs[:],
                                               scalar=float(MAXC),
                                               in1=lpos[s_i][:],
                                               op0=OP.mult, op1=OP.add)
                over = bigp.tile([P, NT], mybir.dt.uint8, tag=f"over{s_i}")
                nc.vector.tensor_scalar(out=over[:], in0=lpos[s_i][:],
                                        scalar1=float(MAXC), scalar2=None,
                                        op0=OP.is_ge)
                nc.vector.select(out=loc[:], mask=over[:], on_true=trash_t[:],
                                 on_false=loc[:])
                li = bigp.tile([P, NT], I16, tag=f"loci{s_i}")
                nc.vector.tensor_copy(li[:], loc[:])
                loc_i16.append(li)

            # wrapped scatter / gather index tiles
            w_sb = []
            for s_i in range(2):
                wt = bigp.tile([P, NT, E], I16, tag=f"w{s_i}")
                for c in range(8):
                    nc.sync.dma_start(wt[0:16, :, c], loc_i16[s_i][16 * c:16 * c + 16, :])
                for rep in (16, 32, 64):
                    nc.sync.dma_start(wt[rep:2 * rep], wt[0:rep])
                w_sb.append(wt)
            wg_sb = bigp.tile([P, NT, 16], I16)
            for c in range(16):
                src = loc_i16[0] if c < 8 else loc_i16[1]
                cc = c % 8
                nc.sync.dma_start(wg_sb[0:16, :, c], src[16 * cc:16 * cc + 16, :])
            for rep in (16, 32, 64):
                nc.sync.dma_start(wg_sb[rep:2 * rep], wg_sb[0:rep])

        # =================== dispatch scatter + collective ===================
        x_bf = bigp.tile([P, NT, D], BF16)
        for q in range(4):
            qs = slice(q * (NT // 4), (q + 1) * (NT // 4))
            nc.vector.tensor_copy(x_bf[:, qs, :], x_all[:, qs, :])
        HALF = TOK // 2
        for wsb in w_sb:
            for h in range(2):
                nc.gpsimd.dma_scatter_add(
                    xin_bf[:], x_bf[:, h * (NT // 2):(h + 1) * (NT // 2), :],
                    wsb[:].rearrange("p t e -> p (t e)")[:, h * (HALF // 16):(h + 1) * (HALF // 16)],
                    HALF, HALF, D)

        cout_b = dram.tile([NCORES, E], F32, addr_space="Shared")
        nc.gpsimd.collective_compute(
            "AllGather", OP.bypass,
            ins=[cin_b.opt()], outs=[cout_b.opt()],
            replica_groups=[list(range(NCORES))])
        cnts_sb = consts.tile([NCORES, E], F32)
        nc.sync.dma_start(cnts_sb[:], cout_b[:])

        # zero the trash tile of y_all
        ztile = consts.tile([P, D], BF16)
        nc.vector.memset(ztile[:], 0.0)
        nc.sync.dma_start(y_all[TRASH:TRASH + P, :], ztile[:])

        # =================== expert MLP ===================
        with tc.tile_pool(name="wts", bufs=2) as wts, \
             tc.tile_pool(name="work", bufs=4) as work, \
             tc.tile_pool(name="smp", bufs=6) as smp, \
             tc.tile_pool(name="psB", bufs=1, space="PSUM") as psB:

            ones1 = consts.tile([1, P], BF16)
            nc.vector.memset(ones1[:], 1.0)
            for e in range(E):
                wa = wts.tile([P, 2, D + 1], BF16, tag="wa")
                nc.gpsimd.dma_start(wa[:, :, :D], w1d[e].rearrange("(k p) h -> p k h", p=P))
                wb = wts.tile([P, 2, D + 1], BF16, tag="wb")
                nc.gpsimd.dma_start(wb[:, :, :D], w2d[e].rearrange("(k p) h -> p k h", p=P))
                with nc.allow_low_precision(reason="fp16 row-sum cols"):
                    for k in range(2):
                        nc.vector.tensor_reduce(wa[:, k, D:D + 1], wa[:, k, :D],
                                                axis=AX.X, op=OP.add)
                        nc.vector.tensor_reduce(wb[:, k, D:D + 1], wb[:, k, :D],
                                                axis=AX.X, op=OP.add)
                b1r = wts.tile([1, D + 1], BF16, tag="b1r")
                nc.gpsimd.dma_start(b1r[:, :D], b1d[e][None, :])
                with nc.allow_low_precision(reason="fp16 bias sum col"):
                    nc.vector.tensor_reduce(b1r[:, D:D + 1], b1r[:, :D], axis=AX.X,
                                            op=OP.add)
                b2r = wts.tile([1, D + 1], BF16, tag="b2r")
                nc.gpsimd.dma_start(b2r[:, :D], b2d[e][None, :])
                with nc.allow_low_precision(reason="fp16 bias sum col"):
                    nc.vector.tensor_reduce(b2r[:, D:D + 1], b2r[:, :D], axis=AX.X,
                                            op=OP.add)
                if ln_ident:
                    s1bc = c1bc = s2bc = c2bc = None
                else:
                    s1bc = wts.tile([P, D], BF16, tag="s1bc")
                    nc.gpsimd.dma_start(s1bc[:], s1d[e][None, :].to_broadcast([P, D]))
                    c1bc = wts.tile([P, D], BF16, tag="c1bc")
                    nc.gpsimd.dma_start(c1bc[:], c1d[e][None, :].to_broadcast([P, D]))
                    s2bc = wts.tile([P, D], BF16, tag="s2bc")
                    nc.gpsimd.dma_start(s2bc[:], s2d[e][None, :].to_broadcast([P, D]))
                    c2bc = wts.tile([P, D], BF16, tag="c2bc")
                    nc.gpsimd.dma_start(c2bc[:], c2d[e][None, :].to_broadcast([P, D]))

                def stage1(w):
                    row0 = e * MAXC + w * WV * P
                    xrow = work.tile([P, WV, D], BF16, tag="xrow")
                    nc.scalar.dma_start(
                        xrow[:],
                        xin_bf[row0:row0 + WV * P, :].rearrange("(t p) d -> p t d",
                                                                p=P))
                    xts = work.tile([P, 2, WV * P], BF16, tag="xts")
                    for t in range(WV):
                        for k in range(2):
                            xtp_ps = psB.tile([P, P], BF16, tag=f"ups{t}")
                            nc.tensor.transpose(xtp_ps[:],
                                                xrow[:, t, k * P:(k + 1) * P],
                                                ident16[:])
                            nc.vector.tensor_copy(xts[:, k, t * P:(t + 1) * P],
                                                  xtp_ps[:])
                    h_wav = _mlp_wave(nc, psB, work, smp, eps_t, ones1,
                                      xts, wa, b1r, s1bc, c1bc, relu=True, pfx="u",
                                      ln_ident=ln_ident)
                    hts = work.tile([P, 2, WV * P], BF16, tag="hts")
                    for t in range(WV):
                        for k in range(2):
                            tp_ps = psB.tile([P, P], BF16, tag=f"vps{t}")
                            nc.tensor.transpose(tp_ps[:], h_wav[:, t, k * P:(k + 1) * P],
                                                ident16[:])
                            if k == 0:
                                nc.vector.tensor_copy(hts[:, k, t * P:(t + 1) * P],
                                                      tp_ps[:])
                            else:
                                nc.scalar.copy(hts[:, k, t * P:(t + 1) * P], tp_ps[:])
                    return hts

                def stage2(w, hts):
                    row0 = e * MAXC + w * WV * P
                    y_wav = _mlp_wave(nc, psB, work, smp, eps_t, ones1,
                                      hts, wb, b2r, s2bc, c2bc, relu=False, pfx="v",
                                      ln_ident=ln_ident)
                    nc.scalar.dma_start(
                        y_all[row0:row0 + WV * P, :].rearrange("(t r) d -> r t d",
                                                               r=P),
                        y_wav[:])

                prev = None
                for w in range(ETILES // WV):
                    hts = stage1(w)
                    if prev is not None:
                        stage2(*prev)
                    prev = (w, hts)
                stage2(*prev)

        # =================== combine ===================
        with tc.tile_pool(name="cwk", bufs=2) as work, \
             tc.tile_pool(name="cps", bufs=1, space="PSUM") as psC:

            base_ps = psC.tile([E, 1], F32, tag="ups0")
            nc.tensor.matmul(base_ps[:], lhsT=cnts_sb[:], rhs=mlt_sb[:],
                             start=True, stop=True)
            capq = consts.tile([E, 1], F32)
            nc.vector.tensor_scalar(out=capq[:], in0=base_ps[:], scalar1=-1.0,
                                    scalar2=float(CAP), op0=OP.mult, op1=OP.add)
            cap_ps = psC.tile([1, E], F32, tag="ups1")
            nc.tensor.transpose(cap_ps[:], capq[:], ident[:E, :E])
            cap_row = consts.tile([1, E], F32)
            nc.scalar.copy(cap_row[:], cap_ps[:])
            cap_bc = consts.tile([P, E], F32)
            nc.gpsimd.partition_broadcast(cap_bc[:], cap_row[:])

            gk16 = []
            for s_i, (idxs, lps, gs) in enumerate(
                    ((idx1_sb, lpos[0], g1_sb), (idx2_sb, lpos[1], g2_sb))):
                msk = work.tile([P, NT, E], F32, tag="msk")
                nc.vector.tensor_tensor(
                    out=msk[:], in0=idxs[:, :, None].to_broadcast([P, NT, E]),
                    in1=iota_f[:, None, :].to_broadcast([P, NT, E]), op=OP.is_equal)
                nc.vector.tensor_tensor(
                    out=msk[:], in0=msk[:],
                    in1=cap_bc[:, None, :].to_broadcast([P, NT, E]), op=OP.mult)
                thr = work.tile([P, NT], F32, tag="thr")
                nc.vector.tensor_reduce(thr[:], msk[:], axis=AX.X, op=OP.add)
                kp = work.tile([P, NT], F32, tag="keep")
                nc.vector.tensor_tensor(out=kp[:], in0=lps[:], in1=thr[:],
                                        op=OP.is_lt)
                gkt = bigp.tile([P, NT], BF16, tag=f"gk16_{s_i}")
                nc.vector.tensor_tensor(out=gkt[:], in0=gs[:], in1=kp[:], op=OP.mult)
                gk16.append(gkt)

            CB = 4
            for tb in range(NT // CB):
                cbs = slice(tb * CB, (tb + 1) * CB)
                yg = work.tile([P, CB, 2, D], BF16, tag="yg")
                nc.gpsimd.dma_gather(yg[:].rearrange("p a b d -> p (a b) d"),
                                     y_all[:], wg_sb[:, cbs, :],
                                     CB * 2 * P, CB * 2 * P, D)
                g0 = work.tile([P, CB, D], BF16, tag="g0t")
                nc.vector.tensor_tensor(
                    out=g0[:], in0=yg[:, :, 0, :],
                    in1=gk16[0][:, cbs, None].to_broadcast([P, CB, D]), op=OP.mult)
                g1 = work.tile([P, CB, D], BF16, tag="g1t")
                nc.vector.tensor_tensor(
                    out=g1[:], in0=yg[:, :, 1, :],
                    in1=gk16[1][:, cbs, None].to_broadcast([P, CB, D]), op=OP.mult)
                acc = work.tile([P, CB, D], BF16, tag="acc")
                nc.vector.tensor_tensor(out=acc[:], in0=g0[:], in1=g1[:], op=OP.add)
                nc.vector.tensor_tensor(out=acc[:], in0=acc[:],
                                        in1=x_bf[:, cbs, :], op=OP.add)
                ot = work.tile([P, CB, D], BF16, tag="ot")
                nc.scalar.activation(ot[:], acc[:], ACTF.Relu)
                nc.sync.dma_start(
                    out_o[tb * CB * P:(tb + 1) * CB * P, :].rearrange(
                        "(t r) d -> r t d", r=P),
                    ot[:])

    nc.compile()
    return nc


def _mlp_wave(nc, psB, work, smp, eps_t, ones1, xts, w_sb, b_row, sbc, cbc, relu,
              pfx, ln_ident):
    out_wav = work.tile([P, WV, D], BF16, tag="hwav" if relu else "ywav")
    ups, mus, rstds = [], [], []
    for t in range(WV):
        u_ps = psB.tile([P, D + 1], F32, tag=f"{pfx}ps{t}")
        nc.tensor.matmul(u_ps[:], lhsT=ones1[:], rhs=b_row[:], start=True, stop=False,
                         skip_group_check=True)
        for k in range(2):
            nc.tensor.matmul(u_ps[:], lhsT=xts[:, k, t * P:(t + 1) * P],
                             rhs=w_sb[:, k, :], start=False, stop=(k == 1),
                             skip_group_check=True)
        ups.append(u_ps)
    sqs = []
    for t in range(WV):
        ssq = smp.tile([P, 1], F32, tag=f"{pfx}ssq{t}")
        usq = work.tile([P, D], BF16, tag="usq")
        nc.scalar.activation(usq[:], ups[t][:, :D], ACTF.Square, accum_out=ssq[:])
        sqs.append(ssq)
    for t in range(WV):
        mu = smp.tile([P, 1], F32, tag=f"{pfx}mu{t}")
        nc.vector.tensor_scalar_mul(mu[:], ups[t][:, D:D + 1], 1.0 / D)
        mu2 = smp.tile([P, 1], F32, tag="mu2")
        nc.vector.tensor_tensor(out=mu2[:], in0=mu[:], in1=mu[:], op=OP.mult)
        var = smp.tile([P, 1], F32, tag="var")
        nc.vector.tensor_scalar(out=var[:], in0=sqs[t][:], scalar1=1.0 / D,
                                scalar2=mu2[:], op0=OP.mult, op1=OP.subtract)
        nc.scalar.activation(var[:], var[:], ACTF.Sqrt, bias=eps_t[:])
        rstd = smp.tile([P, 1], F32, tag=f"{pfx}rstd{t}")
        nc.vector.reciprocal(rstd[:], var[:])
        mus.append(mu)
        rstds.append(rstd)
    for t in range(WV):
        if ln_ident and not relu:
            nc.vector.tensor_scalar(out=out_wav[:, t, :], in0=ups[t][:, :D],
                                    scalar1=mus[t][:], scalar2=rstds[t][:],
                                    op0=OP.subtract, op1=OP.mult)
            continue
        z = work.tile([P, D], BF16, tag="z")
        nc.vector.tensor_scalar(out=z[:], in0=ups[t][:, :D], scalar1=mus[t][:],
                                scalar2=rstds[t][:], op0=OP.subtract, op1=OP.mult)
        if ln_ident:
            nc.scalar.activation(out_wav[:, t, :], z[:], ACTF.Relu)
            continue
        t1 = work.tile([P, D], BF16, tag="t1")
        nc.vector.tensor_tensor(out=t1[:], in0=z[:], in1=sbc[:], op=OP.mult)
        if relu:
            hp = work.tile([P, D], BF16, tag="hp")
            nc.vector.tensor_tensor(out=hp[:], in0=t1[:], in1=cbc[:], op=OP.add)
            nc.scalar.activation(out_wav[:, t, :], hp[:], ACTF.Relu)
        else:
            nc.vector.tensor_tensor(out=out_wav[:, t, :], in0=t1[:], in1=cbc[:],
                                    op=OP.add)
    return out_wav


_CACHE = {}


def _program(ln_ident=True):
    if ln_ident not in _CACHE:
        _CACHE[ln_ident] = build_fused(ln_ident)
    return _CACHE[ln_ident]


def _run(nc, x0, weights, **kw):
    in_maps = []
    for c in range(NCORES):
        m = {
            "x": np.ascontiguousarray(x0[c * TOK:(c + 1) * TOK]),
            "mask_lt": (np.arange(NCORES) < c).astype(np.float32)[:, None],
        }
        m.update(weights)
        in_maps.append(m)
    return run_bass_kernel_spmd(nc, in_maps, core_ids=list(range(NCORES)), **kw)


def kernel(x0, Wr, br, W1, b1, ln1_s, ln1_b, W2, b2, ln2_s, ln2_b,
           _collect_times=None):
    ln_ident = bool(
        np.all(np.asarray(ln1_s) == 1.0) and np.all(np.asarray(ln1_b) == 0.0)
        and np.all(np.asarray(ln2_s) == 1.0) and np.all(np.asarray(ln2_b) == 0.0))
    nc = _program(ln_ident)
    x0 = np.ascontiguousarray(np.asarray(x0, np.float32))
    weights = {
        "wr": np.asarray(Wr, np.float32), "br": np.asarray(br, np.float32),
        "w1d": np.asarray(W1, np.float32), "b1d": np.asarray(b1, np.float32),
        "s1d": np.asarray(ln1_s, np.float32), "c1d": np.asarray(ln1_b, np.float32),
        "w2d": np.asarray(W2, np.float32), "b2d": np.asarray(b2, np.float32),
        "s2d": np.asarray(ln2_s, np.float32), "c2d": np.asarray(ln2_b, np.float32),
    }
    res = _run(nc, x0, weights)
    out = np.concatenate(
        [res.results[c]["out"].astype(np.float32) for c in range(NCORES)], axis=0)
    if _collect_times is not None:
        _collect_times.append((res,))
    return out


# revision 5
# speedup vs baseline: 1.0177x; 1.0177x over previous
"""Trainium2 Bass kernel for nn_MoEResBlock — fused single-launch version.

Per core (8192 tokens): router -> top-2 + gates -> hierarchical cumsum
positions -> SWDGE scatter into per-(core,expert) DRAM regions -> on-device
AllGather of per-core expert counts (overlapped with the expert MLP) ->
dense per-expert MLP (fp16 matmuls, PE transposes, LN via augmented mean
column) -> SWDGE gather-combine with exact global-capacity gates ->
residual + relu -> bf16 out (host upconverts).
"""

import sys

for _p in ("/opt/trn_rl_repo",):
    if _p not in sys.path:
        sys.path.insert(0, _p)

from contextlib import ExitStack

import numpy as np

import concourse.bass as bass
import concourse.mybir as mybir
import concourse.tile as tile
from concourse import bacc
from concourse.bass_utils import run_bass_kernel_spmd
from concourse.masks import make_identity

F32 = mybir.dt.float32
I16 = mybir.dt.int16
I32 = mybir.dt.int32
BF16 = mybir.dt.float16  # half dtype for matmul operands (fp16: 10-bit mantissa)
AX = mybir.AxisListType
OP = mybir.AluOpType
ACTF = mybir.ActivationFunctionType

P = 128
D = 256
E = 8
NCORES = 8
TOK = 65536 // NCORES
NT = TOK // P
GRP = 8
MAXC = 2560
ETILES = MAXC // P
WV = 2
TRASH = E * MAXC
XROWS = TRASH + P
CAP = 16384
BIG = 1000.0
NEG = -1.0e30
LN_EPS = 1e-6


def build_fused(ln_ident=True):
    nc = bacc.Bacc("TRN2", target_bir_lowering=False, debug=False)

    x = nc.dram_tensor("x", [TOK, D], F32, kind="ExternalInput")
    wr = nc.dram_tensor("wr", [D, E], F32, kind="ExternalInput")
    br = nc.dram_tensor("br", [E], F32, kind="ExternalInput")
    w1d = nc.dram_tensor("w1d", [E, D, D], F32, kind="ExternalInput")
    b1d = nc.dram_tensor("b1d", [E, D], F32, kind="ExternalInput")
    s1d = nc.dram_tensor("s1d", [E, D], F32, kind="ExternalInput")
    c1d = nc.dram_tensor("c1d", [E, D], F32, kind="ExternalInput")
    w2d = nc.dram_tensor("w2d", [E, D, D], F32, kind="ExternalInput")
    b2d = nc.dram_tensor("b2d", [E, D], F32, kind="ExternalInput")
    s2d = nc.dram_tensor("s2d", [E, D], F32, kind="ExternalInput")
    c2d = nc.dram_tensor("c2d", [E, D], F32, kind="ExternalInput")
    mask_lt = nc.dram_tensor("mask_lt", [NCORES, 1], F32, kind="ExternalInput")

    out_o = nc.dram_tensor("out", [TOK, D], BF16, kind="ExternalOutput")
    # scatter-add target: ExternalOutput => guaranteed zero-initialized
    xin_bf = nc.dram_tensor("xin", [XROWS, D], BF16, kind="ExternalOutput")
    y_all = nc.dram_tensor("y_all", [XROWS, D], BF16)

    with tile.TileContext(nc) as tc, ExitStack() as ctx:
        consts = ctx.enter_context(tc.tile_pool(name="consts", bufs=1))
        bigp = ctx.enter_context(tc.tile_pool(name="bigp", bufs=1))
        dram = ctx.enter_context(tc.tile_pool(name="dram", bufs=2, space="DRAM"))

        ident = consts.tile([P, P], F32)
        make_identity(nc, ident[:])
        ident16 = consts.tile([P, P], BF16)
        nc.vector.tensor_copy(ident16[:], ident[:])
        # SL[p, i] = 1.0 iff p < i
        sl_ci = consts.tile([P, P], I32)
        nc.gpsimd.iota(sl_ci[:], pattern=[[1, P]], base=0, channel_multiplier=0)
        sl_ri = consts.tile([P, P], I32)
        nc.gpsimd.iota(sl_ri[:], pattern=[[0, P]], base=0, channel_multiplier=1)
        sl_c = consts.tile([P, P], F32)
        nc.vector.tensor_copy(sl_c[:], sl_ci[:])
        sl_r = consts.tile([P, P], F32)
        nc.vector.tensor_copy(sl_r[:], sl_ri[:])
        sl = consts.tile([P, P], F32)
        nc.vector.tensor_tensor(out=sl[:], in0=sl_r[:], in1=sl_c[:], op=OP.is_lt)
        iota_i = consts.tile([P, E], I32)
        nc.gpsimd.iota(iota_i[:], pattern=[[1, E]], base=0, channel_multiplier=0)
        iota_f = consts.tile([P, E], F32)
        nc.vector.tensor_copy(iota_f[:], iota_i[:])
        iota_mb = consts.tile([P, E], F32)
        nc.vector.tensor_scalar_add(iota_mb[:], iota_i[:], -BIG)
        ones_col = consts.tile([P, 1], F32)
        nc.vector.memset(ones_col[:], 1.0)
        eps_t = consts.tile([P, 1], F32)
        nc.vector.memset(eps_t[:], LN_EPS)

        br_row = consts.tile([1, E], F32)
        nc.sync.dma_start(br_row[:], br[None, :])
        br_bc = consts.tile([P, E], F32)
        nc.gpsimd.partition_broadcast(br_bc[:], br_row[:])
        wr_sb = consts.tile([P, 2, E], F32)
        nc.sync.dma_start(wr_sb[:], wr.rearrange("(k p) e -> p k e", p=P))
        mlt_sb = consts.tile([NCORES, 1], F32)
        nc.sync.dma_start(mlt_sb[:], mask_lt[:])

        # ---- bulk x load (single read, reused by router/scatter/combine) ----
        x_all = bigp.tile([P, NT, D], F32)
        nc.sync.dma_start(x_all[:], x.rearrange("(t p) d -> p t d", p=P))

        s_all = bigp.tile([P, NT, E], F32)
        m1_all = bigp.tile([P, NT, E], F32)
        m2_all = bigp.tile([P, NT, E], F32)
        idx1_sb = bigp.tile([P, NT], F32)
        idx2_sb = bigp.tile([P, NT], F32)
        g1_sb = bigp.tile([P, NT], F32)
        g2_sb = bigp.tile([P, NT], F32)

        # =================== router ===================
        with tc.tile_pool(name="rxp", bufs=3) as xp, \
             tc.tile_pool(name="rtp", bufs=3) as tp, \
             tc.tile_pool(name="rsm", bufs=4) as sm, \
             tc.tile_pool(name="rps", bufs=2, space="PSUM") as ps, \
             tc.tile_pool(name="rpl", bufs=1, space="PSUM") as pl:

            for g in range(NT // GRP):
                lg = tp.tile([P, GRP, E], F32)
                for t in range(GRP):
                    ti = g * GRP + t
                    xts = tp.tile([P, 2, P], F32, tag="xts")
                    for k in range(2):
                        xt_ps = ps.tile([P, P], F32)
                        nc.tensor.transpose(xt_ps[:], x_all[:, ti, k * P:(k + 1) * P],
                                            ident[:])
                        nc.scalar.copy(xts[:, k, :], xt_ps[:])
                    lg_ps = ps.tile([P, E], F32, tag="lgps")
                    for k in range(2):
                        nc.tensor.matmul(lg_ps[:], lhsT=xts[:, k, :],
                                         rhs=wr_sb[:, k, :],
                                         start=(k == 0), stop=(k == 1))
                    nc.vector.tensor_add(lg[:, t, :], lg_ps[:], br_bc[:])

                gb = slice(g * GRP, (g + 1) * GRP)
                iota_b = iota_mb[:, None, :].to_broadcast([P, GRP, E])
                m1 = sm.tile([P, GRP, 1], F32)
                nc.vector.tensor_reduce(m1[:], lg[:], axis=AX.X, op=OP.max)
                eq1 = tp.tile([P, GRP, E], F32, tag="eq")
                nc.vector.tensor_tensor(out=eq1[:], in0=lg[:],
                                        in1=m1[:].to_broadcast([P, GRP, E]),
                                        op=OP.is_equal)
                cand = tp.tile([P, GRP, E], F32, tag="cand")
                nc.vector.tensor_tensor(out=cand[:], in0=eq1[:], in1=iota_b,
                                        op=OP.mult)
                i1m = sm.tile([P, GRP, 1], F32)
                nc.vector.tensor_reduce(i1m[:], cand[:], axis=AX.X, op=OP.min)
                nc.vector.tensor_scalar_add(idx1_sb[:, gb], i1m[:, :, 0], BIG)
                nc.vector.tensor_tensor(out=m1_all[:, gb, :], in0=iota_b,
                                        in1=i1m[:].to_broadcast([P, GRP, E]),
                                        op=OP.is_equal)
                l2 = tp.tile([P, GRP, E], F32, tag="l2")
                nc.vector.scalar_tensor_tensor(out=l2[:], in0=m1_all[:, gb, :],
                                               scalar=NEG, in1=lg[:],
                                               op0=OP.mult, op1=OP.add)
                m2 = sm.tile([P, GRP, 1], F32)
                nc.vector.tensor_reduce(m2[:], l2[:], axis=AX.X, op=OP.max)
                eq2 = tp.tile([P, GRP, E], F32, tag="eq")
                nc.vector.tensor_tensor(out=eq2[:], in0=l2[:],
                                        in1=m2[:].to_broadcast([P, GRP, E]),
                                        op=OP.is_equal)
                cand2 = tp.tile([P, GRP, E], F32, tag="cand")
                nc.vector.tensor_tensor(out=cand2[:], in0=eq2[:], in1=iota_b,
                                        op=OP.mult)
                i2m = sm.tile([P, GRP, 1], F32)
                nc.vector.tensor_reduce(i2m[:], cand2[:], axis=AX.X, op=OP.min)
                nc.vector.tensor_scalar_add(idx2_sb[:, gb], i2m[:, :, 0], BIG)
                nc.vector.tensor_tensor(out=m2_all[:, gb, :], in0=iota_b,
                                        in1=i2m[:].to_broadcast([P, GRP, E]),
                                        op=OP.is_equal)
                nc.vector.tensor_tensor(out=s_all[:, gb, :], in0=m1_all[:, gb, :],
                                        in1=m2_all[:, gb, :], op=OP.add)
                dsc = sm.tile([P, GRP, 1], F32)
                nc.vector.tensor_tensor(out=dsc[:], in0=m2[:], in1=m1[:],
                                        op=OP.subtract)
                edv = sm.tile([P, GRP, 1], F32)
                nc.scalar.activation(edv[:], dsc[:], ACTF.Exp)
                nc.vector.tensor_scalar_add(edv[:], edv[:], 1.0)
                g1t = sm.tile([P, GRP, 1], F32)
                nc.vector.reciprocal(g1t[:], edv[:])
                nc.vector.tensor_copy(g1_sb[:, gb], g1t[:, :, 0])
                nc.vector.tensor_scalar(out=g2_sb[:, gb], in0=g1t[:, :, 0],
                                        scalar1=-1.0, scalar2=1.0,
                                        op0=OP.mult, op1=OP.add)

            # ------- hierarchical exclusive cumsum over pair order -------
            s_flat = s_all[:].rearrange("p t e -> p (t e)")
            cab_ps = pl.tile([P, NT * E], F32)
            nc.tensor.matmul(cab_ps[:], lhsT=sl[:], rhs=s_flat, start=True, stop=True)
            cab_sb = bigp.tile([P, NT, E], F32)
            nc.scalar.copy(cab_sb[:].rearrange("p t e -> p (t e)"), cab_ps[:])

            trow_ps = pl.tile([1, NT * E], F32, tag="trow")
            nc.tensor.matmul(trow_ps[:], lhsT=ones_col[:], rhs=s_flat,
                             start=True, stop=True)
            trow_sb = sm.tile([1, NT * E], F32, tag="trowsb")
            nc.scalar.copy(trow_sb[:], trow_ps[:])
            t_p = sm.tile([NT, E], F32, tag="tp64")
            nc.sync.dma_start(t_p[:], trow_sb[:])
            toff_ps = pl.tile([NT, E], F32, tag="toffps")
            nc.tensor.matmul(toff_ps[:], lhsT=sl[:NT, :NT], rhs=t_p[:],
                             start=True, stop=True)
            toff_sb = sm.tile([NT, E], F32, tag="toffsb")
            nc.scalar.copy(toff_sb[:], toff_ps[:])
            toff_row = sm.tile([1, NT * E], F32, tag="toffrow")
            nc.sync.dma_start(toff_row[:], toff_sb[:])
            toff_bc = bigp.tile([P, NT, E], F32)
            nc.gpsimd.partition_broadcast(toff_bc[:].rearrange("p t e -> p (t e)"),
                                          toff_row[:])
            nc.vector.tensor_add(cab_sb[:], cab_sb[:], toff_bc[:])

            cnt_ps = pl.tile([1, E], F32, tag="cntps")
            nc.tensor.matmul(cnt_ps[:], lhsT=ones_col[:NT, :], rhs=t_p[:],
                             start=True, stop=True)
            cnt_sb = sm.tile([1, E], F32, tag="cntsb")
            nc.scalar.copy(cnt_sb[:], cnt_ps[:])
            # collective input bounce
            cin_b = dram.tile([1, E], F32)
            nc.sync.dma_start(cin_b[:], cnt_sb[:])

            # ------- per-pair local positions + dispatch locations -------
            tmp = bigp.tile([P, NT, E], F32)
            lpos = [None, None]
            for s_i, mask in ((0, m1_all), (1, m2_all)):
                nc.vector.tensor_tensor(out=tmp[:], in0=mask[:], in1=cab_sb[:],
                                        op=OP.mult)
                lp = bigp.tile([P, NT], F32, tag=f"lpos{s_i}")
                nc.vector.tensor_reduce(lp[:], tmp[:], axis=AX.X, op=OP.add)
                lpos[s_i] = lp

            trash_t = consts.tile([P, NT], F32)
            nc.vector.memset(trash_t[:], float(TRASH))
            loc_i16 = []
            for s_i, idxs in ((0, idx1_sb), (1, idx2_sb)):
                loc = bigp.tile([P, NT], F32, tag=f"loc{s_i}")
                nc.vector.scalar_tensor_tensor(out=loc[:], in0=idxs[:],
                                               scalar=float(MAXC),
                                               in1=lpos[s_i][:],
                                               op0=OP.mult, op1=OP.add)
                over = bigp.tile([P, NT], mybir.dt.uint8, tag=f"over{s_i}")
                nc.vector.tensor_scalar(out=over[:], in0=lpos[s_i][:],
                                        scalar1=float(MAXC), scalar2=None,
                                        op0=OP.is_ge)
                nc.vector.select(out=loc[:], mask=over[:], on_true=trash_t[:],
                                 on_false=loc[:])
                li = bigp.tile([P, NT], I16, tag=f"loci{s_i}")
                nc.vector.tensor_copy(li[:], loc[:])
                loc_i16.append(li)

            # wrapped scatter / gather index tiles
            w_sb = []
            for s_i in range(2):
                wt = bigp.tile([P, NT, E], I16, tag=f"w{s_i}")
                for c in range(8):
                    nc.sync.dma_start(wt[0:16, :, c], loc_i16[s_i][16 * c:16 * c + 16, :])
                for rep in (16, 32, 64):
                    nc.sync.dma_start(wt[rep:2 * rep], wt[0:rep])
                w_sb.append(wt)
            wg_sb = bigp.tile([P, NT, 16], I16)
            for c in range(16):
                src = loc_i16[0] if c < 8 else loc_i16[1]
                cc = c % 8
                nc.sync.dma_start(wg_sb[0:16, :, c], src[16 * cc:16 * cc + 16, :])
            for rep in (16, 32, 64):
                nc.sync.dma_start(wg_sb[rep:2 * rep], wg_sb[0:rep])

        # =================== dispatch scatter + collective ===================
        x_bf = bigp.tile([P, NT, D], BF16)
        for q in range(4):
            qs = slice(q * (NT // 4), (q + 1) * (NT // 4))
            nc.vector.tensor_copy(x_bf[:, qs, :], x_all[:, qs, :])
        HALF = TOK // 2
        for wsb in w_sb:
            for h in range(2):
                nc.gpsimd.dma_scatter_add(
                    xin_bf[:], x_bf[:, h * (NT // 2):(h + 1) * (NT // 2), :],
                    wsb[:].rearrange("p t e -> p (t e)")[:, h * (HALF // 16):(h + 1) * (HALF // 16)],
                    HALF, HALF, D)

        cout_b = dram.tile([NCORES, E], F32, addr_space="Shared")
        nc.gpsimd.collective_compute(
            "AllGather", OP.bypass,
            ins=[cin_b.opt()], outs=[cout_b.opt()],
            replica_groups=[list(range(NCORES))])
        cnts_sb = consts.tile([NCORES, E], F32)
        nc.sync.dma_start(cnts_sb[:], cout_b[:])

        # zero the trash tile of y_all
        ztile = consts.tile([P, D], BF16)
        nc.vector.memset(ztile[:], 0.0)
        nc.sync.dma_start(y_all[TRASH:TRASH + P, :], ztile[:])

        # =================== expert MLP ===================
        with tc.tile_pool(name="wts", bufs=2) as wts, \
             tc.tile_pool(name="work", bufs=4) as work, \
             tc.tile_pool(name="smp", bufs=6) as smp, \
             tc.tile_pool(name="psB", bufs=1, space="PSUM") as psB:

            ones1 = consts.tile([1, P], BF16)
            nc.vector.memset(ones1[:], 1.0)
            for e in range(E):
                wa = wts.tile([P, 2, D + 1], BF16, tag="wa")
                nc.gpsimd.dma_start(wa[:, :, :D], w1d[e].rearrange("(k p) h -> p k h", p=P))
                wb = wts.tile([P, 2, D + 1], BF16, tag="wb")
                nc.gpsimd.dma_start(wb[:, :, :D], w2d[e].rearrange("(k p) h -> p k h", p=P))
                with nc.allow_low_precision(reason="fp16 row-sum cols"):
                    for k in range(2):
                        nc.vector.tensor_reduce(wa[:, k, D:D + 1], wa[:, k, :D],
                                                axis=AX.X, op=OP.add)
                        nc.vector.tensor_reduce(wb[:, k, D:D + 1], wb[:, k, :D],
                                                axis=AX.X, op=OP.add)
                b1r = wts.tile([1, D + 1], BF16, tag="b1r")
                nc.gpsimd.dma_start(b1r[:, :D], b1d[e][None, :])
                with nc.allow_low_precision(reason="fp16 bias sum col"):
                    nc.vector.tensor_reduce(b1r[:, D:D + 1], b1r[:, :D], axis=AX.X,
                                            op=OP.add)
                b2r = wts.tile([1, D + 1], BF16, tag="b2r")
                nc.gpsimd.dma_start(b2r[:, :D], b2d[e][None, :])
                with nc.allow_low_precision(reason="fp16 bias sum col"):
                    nc.vector.tensor_reduce(b2r[:, D:D + 1], b2r[:, :D], axis=AX.X,
                                            op=OP.add)
                if ln_ident:
                    s1bc = c1bc = s2bc = c2bc = None
                else:
                    s1bc = wts.tile([P, D], BF16, tag="s1bc")
                    nc.gpsimd.dma_start(s1bc[:], s1d[e][None, :].to_broadcast([P, D]))
                    c1bc = wts.tile([P, D], BF16, tag="c1bc")
                    nc.gpsimd.dma_start(c1bc[:], c1d[e][None, :].to_broadcast([P, D]))
                    s2bc = wts.tile([P, D], BF16, tag="s2bc")
                    nc.gpsimd.dma_start(s2bc[:], s2d[e][None, :].to_broadcast([P, D]))
                    c2bc = wts.tile([P, D], BF16, tag="c2bc")
                    nc.gpsimd.dma_start(c2bc[:], c2d[e][None, :].to_broadcast([P, D]))

                def stage1(w):
                    row0 = e * MAXC + w * WV * P
                    xrow = work.tile([P, WV, D], BF16, tag="xrow")
                    nc.scalar.dma_start(
                        xrow[:],
                        xin_bf[row0:row0 + WV * P, :].rearrange("(t p) d -> p t d",
                                                                p=P))
                    xts = work.tile([P, 2, WV * P], BF16, tag="xts")
                    for t in range(WV):
                        for k in range(2):
                            xtp_ps = psB.tile([P, P], BF16, tag=f"xtp{t}")
                            nc.tensor.transpose(xtp_ps[:],
                                                xrow[:, t, k * P:(k + 1) * P],
                                                ident16[:])
                            nc.vector.tensor_copy(xts[:, k, t * P:(t + 1) * P],
                                                  xtp_ps[:])
                    h_wav = _mlp_wave(nc, psB, work, smp, eps_t, ones1,
                                      xts, wa, b1r, s1bc, c1bc, relu=True, pfx="u",
                                      ln_ident=ln_ident)
                    hts = work.tile([P, 2, WV * P], BF16, tag="hts")
                    for t in range(WV):
                        for k in range(2):
                            tp_ps = psB.tile([P, P], BF16, tag=f"htp{t}")
                            nc.tensor.transpose(tp_ps[:], h_wav[:, t, k * P:(k + 1) * P],
                                                ident16[:])
                            if k == 0:
                                nc.vector.tensor_copy(hts[:, k, t * P:(t + 1) * P],
                                                      tp_ps[:])
                            else:
                                nc.scalar.copy(hts[:, k, t * P:(t + 1) * P], tp_ps[:])
                    return hts

                def stage2(w, hts):
                    row0 = e * MAXC + w * WV * P
                    y_wav = _mlp_wave(nc, psB, work, smp, eps_t, ones1,
                                      hts, wb, b2r, s2bc, c2bc, relu=False, pfx="v",
                                      ln_ident=ln_ident)
                    nc.scalar.dma_start(
                        y_all[row0:row0 + WV * P, :].rearrange("(t r) d -> r t d",
                                                               r=P),
                        y_wav[:])

                prev = None
                for w in range(ETILES // WV):
                    hts = stage1(w)
                    if prev is not None:
                        stage2(*prev)
                    prev = (w, hts)
                stage2(*prev)

        # =================== combine ===================
        with tc.tile_pool(name="cwk", bufs=2) as work, \
             tc.tile_pool(name="cps", bufs=1, space="PSUM") as psC:

            base_ps = psC.tile([E, 1], F32, tag="ups0")
            nc.tensor.matmul(base_ps[:], lhsT=cnts_sb[:], rhs=mlt_sb[:],
                             start=True, stop=True)
            capq = consts.tile([E, 1], F32)
            nc.vector.tensor_scalar(out=capq[:], in0=base_ps[:], scalar1=-1.0,
                                    scalar2=float(CAP), op0=OP.mult, op1=OP.add)
            cap_ps = psC.tile([1, E], F32, tag="ups1")
            nc.tensor.transpose(cap_ps[:], capq[:], ident[:E, :E])
            cap_row = consts.tile([1, E], F32)
            nc.scalar.copy(cap_row[:], cap_ps[:])
            cap_bc = consts.tile([P, E], F32)
            nc.gpsimd.partition_broadcast(cap_bc[:], cap_row[:])

            gk16 = []
            for s_i, (idxs, lps, gs) in enumerate(
                    ((idx1_sb, lpos[0], g1_sb), (idx2_sb, lpos[1], g2_sb))):
                msk = work.tile([P, NT, E], F32, tag="msk")
                nc.vector.tensor_tensor(
                    out=msk[:], in0=idxs[:, :, None].to_broadcast([P, NT, E]),
                    in1=iota_f[:, None, :].to_broadcast([P, NT, E]), op=OP.is_equal)
                nc.vector.tensor_tensor(
                    out=msk[:], in0=msk[:],
                    in1=cap_bc[:, None, :].to_broadcast([P, NT, E]), op=OP.mult)
                thr = work.tile([P, NT], F32, tag="thr")
                nc.vector.tensor_reduce(thr[:], msk[:], axis=AX.X, op=OP.add)
                kp = work.tile([P, NT], F32, tag="keep")
                nc.vector.tensor_tensor(out=kp[:], in0=lps[:], in1=thr[:],
                                        op=OP.is_lt)
                gkt = bigp.tile([P, NT], BF16, tag=f"gk16_{s_i}")
                nc.vector.tensor_tensor(out=gkt[:], in0=gs[:], in1=kp[:], op=OP.mult)
                gk16.append(gkt)

            CB = 4
            for tb in range(NT // CB):
                cbs = slice(tb * CB, (tb + 1) * CB)
                yg = work.tile([P, CB, 2, D], BF16, tag="yg")
                nc.gpsimd.dma_gather(yg[:].rearrange("p a b d -> p (a b) d"),
                                     y_all[:], wg_sb[:, cbs, :],
                                     CB * 2 * P, CB * 2 * P, D)
                g0 = work.tile([P, CB, D], BF16, tag="g0t")
                nc.vector.tensor_tensor(
                    out=g0[:], in0=yg[:, :, 0, :],
                    in1=gk16[0][:, cbs, None].to_broadcast([P, CB, D]), op=OP.mult)
                g1 = work.tile([P, CB, D], BF16, tag="g1t")
                nc.vector.tensor_tensor(
                    out=g1[:], in0=yg[:, :, 1, :],
                    in1=gk16[1][:, cbs, None].to_broadcast([P, CB, D]), op=OP.mult)
                acc = work.tile([P, CB, D], BF16, tag="acc")
                nc.vector.tensor_tensor(out=acc[:], in0=g0[:], in1=g1[:], op=OP.add)
                nc.vector.tensor_tensor(out=acc[:], in0=acc[:],
                                        in1=x_bf[:, cbs, :], op=OP.add)
                ot = work.tile([P, CB, D], BF16, tag="ot")
                nc.scalar.activation(ot[:], acc[:], ACTF.Relu)
                nc.sync.dma_start(
                    out_o[tb * CB * P:(tb + 1) * CB * P, :].rearrange(
                        "(t r) d -> r t d", r=P),
                    ot[:])

    nc.compile()
    return nc


def _mlp_wave(nc, psB, work, smp, eps_t, ones1, xts, w_sb, b_row, sbc, cbc, relu,
              pfx, ln_ident):
    out_wav = work.tile([P, WV, D], BF16, tag="hwav" if relu else "ywav")
    ups, mus, rstds = [], [], []
    for t in range(WV):
        u_ps = psB.tile([P, D + 1], F32, tag=f"{pfx}ps{t}")
        nc.tensor.matmul(u_ps[:], lhsT=ones1[:], rhs=b_row[:], start=True, stop=False,
                         skip_group_check=True)
        for k in range(2):
            nc.tensor.matmul(u_ps[:], lhsT=xts[:, k, t * P:(t + 1) * P],
                             rhs=w_sb[:, k, :], start=False, stop=(k == 1),
                             skip_group_check=True)
        ups.append(u_ps)
    sqs = []
    for t in range(WV):
        ssq = smp.tile([P, 1], F32, tag=f"{pfx}ssq{t}")
        usq = work.tile([P, D], BF16, tag="usq")
        nc.scalar.activation(usq[:], ups[t][:, :D], ACTF.Square, accum_out=ssq[:])
        sqs.append(ssq)
    for t in range(WV):
        mu = smp.tile([P, 1], F32, tag=f"{pfx}mu{t}")
        nc.vector.tensor_scalar_mul(mu[:], ups[t][:, D:D + 1], 1.0 / D)
        mu2 = smp.tile([P, 1], F32, tag="mu2")
        nc.vector.tensor_tensor(out=mu2[:], in0=mu[:], in1=mu[:], op=OP.mult)
        var = smp.tile([P, 1], F32, tag="var")
        nc.vector.tensor_scalar(out=var[:], in0=sqs[t][:], scalar1=1.0 / D,
                                scalar2=mu2[:], op0=OP.mult, op1=OP.subtract)
        nc.scalar.activation(var[:], var[:], ACTF.Sqrt, bias=eps_t[:])
        rstd = smp.tile([P, 1], F32, tag=f"{pfx}rstd{t}")
        nc.vector.reciprocal(rstd[:], var[:])
        mus.append(mu)
        rstds.append(rstd)
    for t in range(WV):
        if ln_ident and not relu:
            nc.vector.tensor_scalar(out=out_wav[:, t, :], in0=ups[t][:, :D],
                                    scalar1=mus[t][:], scalar2=rstds[t][:],
                                    op0=OP.subtract, op1=OP.mult)
            continue
        z = work.tile([P, D], BF16, tag="z")
        nc.vector.tensor_scalar(out=z[:], in0=ups[t][:, :D], scalar1=mus[t][:],
                                scalar2=rstds[t][:], op0=OP.subtract, op1=OP.mult)
        if ln_ident:
            nc.scalar.activation(out_wav[:, t, :], z[:], ACTF.Relu)
            continue
        t1 = work.tile([P, D], BF16, tag="t1")
        nc.vector.tensor_tensor(out=t1[:], in0=z[:], in1=sbc[:], op=OP.mult)
        if relu:
            hp = work.tile([P, D], BF16, tag="hp")
            nc.vector.tensor_tensor(out=hp[:], in0=t1[:], in1=cbc[:], op=OP.add)
            nc.scalar.activation(out_wav[:, t, :], hp[:], ACTF.Relu)
        else:
            nc.vector.tensor_tensor(out=out_wav[:, t, :], in0=t1[:], in1=cbc[:],
                                    op=OP.add)
    return out_wav


_CACHE = {}


def _program(ln_ident=True):
    if ln_ident not in _CACHE:
        _CACHE[ln_ident] = build_fused(ln_ident)
    return _CACHE[ln_ident]


def _run(nc, x0, weights, **kw):
    in_maps = []
    for c in range(NCORES):
        m = {
            "x": np.ascontiguousarray(x0[c * TOK:(c + 1) * TOK]),
            "mask_lt": (np.arange(NCORES) < c).astype(np.float32)[:, None],
        }
        m.update(weights)
        in_maps.append(m)
    return run_bass_kernel_spmd(nc, in_maps, core_ids=list(range(NCORES)), **kw)


def kernel(x0, Wr, br, W1, b1, ln1_s, ln1_b, W2, b2, ln2_s, ln2_b,
           _collect_times=None):
    ln_ident = bool(
        np.all(np.asarray(ln1_s) == 1.0) and np.all(np.asarray(ln1_b) == 0.0)
        and np.all(np.asarray(ln2_s) == 1.0) and np.all(np.asarray(ln2_b) == 0.0))
    nc = _program(ln_ident)
    x0 = np.ascontiguousarray(np.asarray(x0, np.float32))
    weights = {
        "wr": np.asarray(Wr, np.float32), "br": np.asarray(br, np.float32),
        "w1d": np.asarray(W1, np.float32), "b1d": np.asarray(b1, np.float32),
        "s1d": np.asarray(ln1_s, np.float32), "c1d": np.asarray(ln1_b, np.float32),
        "w2d": np.asarray(W2, np.float32), "b2d": np.asarray(b2, np.float32),
        "s2d": np.asarray(ln2_s, np.float32), "c2d": np.asarray(ln2_b, np.float32),
    }
    res = _run(nc, x0, weights)
    out = np.concatenate(
        [res.results[c]["out"].astype(np.float32) for c in range(NCORES)], axis=0)
    if _collect_times is not None:
        _collect_times.append((res,))
    return out


# revision 9
# speedup vs baseline: 1.0886x; 1.0696x over previous
"""Trainium2 Bass kernel for nn_MoEResBlock — fused single-launch version.

Per core (8192 tokens): router -> top-2 + gates -> hierarchical cumsum
positions -> SWDGE scatter into per-(core,expert) DRAM regions -> on-device
AllGather of per-core expert counts (overlapped with the expert MLP) ->
dense per-expert MLP (fp16 matmuls, PE transposes, LN via augmented mean
column) -> SWDGE gather-combine with exact global-capacity gates ->
residual + relu -> bf16 out (host upconverts).
"""

import sys

for _p in ("/opt/trn_rl_repo",):
    if _p not in sys.path:
        sys.path.insert(0, _p)

from contextlib import ExitStack

import numpy as np

import concourse.bass as bass
import concourse.mybir as mybir
import concourse.tile as tile
from concourse import bacc
from concourse.bass_utils import run_bass_kernel_spmd
from concourse.masks import make_identity

F32 = mybir.dt.float32
I16 = mybir.dt.int16
I32 = mybir.dt.int32
BF16 = mybir.dt.float16  # half dtype for matmul operands (fp16: 10-bit mantissa)
AX = mybir.AxisListType
OP = mybir.AluOpType
ACTF = mybir.ActivationFunctionType

P = 128
D = 256
E = 8
NCORES = 8
TOK = 65536 // NCORES
NT = TOK // P
GRP = 8
MAXC = 2560
ETILES = MAXC // P
WV = 2
TRASH = E * MAXC
XROWS = TRASH + P
CAP = 16384
BIG = 1000.0
NEG = -1.0e30
LN_EPS = 1e-6


def build_fused(ln_ident=True):
    nc = bacc.Bacc("TRN2", target_bir_lowering=False, debug=False)

    x = nc.dram_tensor("x", [TOK, D], F32, kind="ExternalInput")
    wr = nc.dram_tensor("wr", [D, E], F32, kind="ExternalInput")
    br = nc.dram_tensor("br", [E], F32, kind="ExternalInput")
    w1d = nc.dram_tensor("w1d", [E, D, D], F32, kind="ExternalInput")
    b1d = nc.dram_tensor("b1d", [E, D], F32, kind="ExternalInput")
    s1d = nc.dram_tensor("s1d", [E, D], F32, kind="ExternalInput")
    c1d = nc.dram_tensor("c1d", [E, D], F32, kind="ExternalInput")
    w2d = nc.dram_tensor("w2d", [E, D, D], F32, kind="ExternalInput")
    b2d = nc.dram_tensor("b2d", [E, D], F32, kind="ExternalInput")
    s2d = nc.dram_tensor("s2d", [E, D], F32, kind="ExternalInput")
    c2d = nc.dram_tensor("c2d", [E, D], F32, kind="ExternalInput")
    mask_lt = nc.dram_tensor("mask_lt", [NCORES, 1], F32, kind="ExternalInput")

    out_o = nc.dram_tensor("out", [TOK, D], BF16, kind="ExternalOutput")
    # scatter-add target: ExternalOutput => guaranteed zero-initialized
    xin_bf = nc.dram_tensor("xin", [XROWS, D], BF16, kind="ExternalOutput")
    y_all = nc.dram_tensor("y_all", [XROWS, D], BF16)

    with tile.TileContext(nc) as tc, ExitStack() as ctx:
        consts = ctx.enter_context(tc.tile_pool(name="consts", bufs=1))
        bigp = ctx.enter_context(tc.tile_pool(name="bigp", bufs=1))
        dram = ctx.enter_context(tc.tile_pool(name="dram", bufs=2, space="DRAM"))

        ident = consts.tile([P, P], F32)
        make_identity(nc, ident[:])
        ident16 = consts.tile([P, P], BF16)
        nc.vector.tensor_copy(ident16[:], ident[:])
        # SL[p, i] = 1.0 iff p < i
        sl_ci = consts.tile([P, P], I32)
        nc.gpsimd.iota(sl_ci[:], pattern=[[1, P]], base=0, channel_multiplier=0)
        sl_ri = consts.tile([P, P], I32)
        nc.gpsimd.iota(sl_ri[:], pattern=[[0, P]], base=0, channel_multiplier=1)
        sl_c = consts.tile([P, P], F32)
        nc.vector.tensor_copy(sl_c[:], sl_ci[:])
        sl_r = consts.tile([P, P], F32)
        nc.vector.tensor_copy(sl_r[:], sl_ri[:])
        sl = consts.tile([P, P], F32)
        nc.vector.tensor_tensor(out=sl[:], in0=sl_r[:], in1=sl_c[:], op=OP.is_lt)
        iota_i = consts.tile([P, E], I32)
        nc.gpsimd.iota(iota_i[:], pattern=[[1, E]], base=0, channel_multiplier=0)
        iota_f = consts.tile([P, E], F32)
        nc.vector.tensor_copy(iota_f[:], iota_i[:])
        iota_mb = consts.tile([P, E], F32)
        nc.vector.tensor_scalar_add(iota_mb[:], iota_i[:], -BIG)
        ones_col = consts.tile([P, 1], F32)
        nc.vector.memset(ones_col[:], 1.0)
        eps_t = consts.tile([P, 1], F32)
        nc.vector.memset(eps_t[:], LN_EPS)

        br_row = consts.tile([1, E], F32)
        nc.sync.dma_start(br_row[:], br[None, :])
        br_bc = consts.tile([P, E], F32)
        nc.gpsimd.partition_broadcast(br_bc[:], br_row[:])
        wr_sb = consts.tile([P, 2, E], F32)
        nc.sync.dma_start(wr_sb[:], wr.rearrange("(k p) e -> p k e", p=P))
        mlt_sb = consts.tile([NCORES, 1], F32)
        nc.sync.dma_start(mlt_sb[:], mask_lt[:])

        # ---- bulk x load (single read, reused by router/scatter/combine) ----
        x_all = bigp.tile([P, NT, D], F32)
        nc.sync.dma_start(x_all[:], x.rearrange("(t p) d -> p t d", p=P))

        s_all = bigp.tile([P, NT, E], F32)
        m1_all = bigp.tile([P, NT, E], F32)
        m2_all = bigp.tile([P, NT, E], F32)
        idx1_sb = bigp.tile([P, NT], F32)
        idx2_sb = bigp.tile([P, NT], F32)
        g1_sb = bigp.tile([P, NT], F32)
        g2_sb = bigp.tile([P, NT], F32)

        # =================== router ===================
        with tc.tile_pool(name="rxp", bufs=3) as xp, \
             tc.tile_pool(name="rtp", bufs=3) as tp, \
             tc.tile_pool(name="rsm", bufs=4) as sm, \
             tc.tile_pool(name="rps", bufs=2, space="PSUM") as ps, \
             tc.tile_pool(name="rpl", bufs=1, space="PSUM") as pl:

            def do_group(g):
                lg = tp.tile([P, GRP, E], F32)
                for t in range(GRP):
                    ti = g * GRP + t
                    xts = tp.tile([P, 2, P], F32, tag="xts")
                    for k in range(2):
                        xt_ps = ps.tile([P, P], F32)
                        nc.tensor.transpose(xt_ps[:], x_all[:, ti, k * P:(k + 1) * P],
                                            ident[:])
                        nc.scalar.copy(xts[:, k, :], xt_ps[:])
                    lg_ps = ps.tile([P, E], F32, tag="lgps")
                    for k in range(2):
                        nc.tensor.matmul(lg_ps[:], lhsT=xts[:, k, :],
                                         rhs=wr_sb[:, k, :],
                                         start=(k == 0), stop=(k == 1))
                    nc.vector.tensor_add(lg[:, t, :], lg_ps[:], br_bc[:])

                gb = slice(g * GRP, (g + 1) * GRP)
                iota_b = iota_mb[:, None, :].to_broadcast([P, GRP, E])
                m1 = sm.tile([P, GRP, 1], F32)
                nc.vector.tensor_reduce(m1[:], lg[:], axis=AX.X, op=OP.max)
                eq1 = tp.tile([P, GRP, E], F32, tag="eq")
                nc.vector.tensor_tensor(out=eq1[:], in0=lg[:],
                                        in1=m1[:].to_broadcast([P, GRP, E]),
                                        op=OP.is_equal)
                cand = tp.tile([P, GRP, E], F32, tag="cand")
                nc.vector.tensor_tensor(out=cand[:], in0=eq1[:], in1=iota_b,
                                        op=OP.mult)
                i1m = sm.tile([P, GRP, 1], F32)
                nc.vector.tensor_reduce(i1m[:], cand[:], axis=AX.X, op=OP.min)
                nc.vector.tensor_scalar_add(idx1_sb[:, gb], i1m[:, :, 0], BIG)
                nc.vector.tensor_tensor(out=m1_all[:, gb, :], in0=iota_b,
                                        in1=i1m[:].to_broadcast([P, GRP, E]),
                                        op=OP.is_equal)
                l2 = tp.tile([P, GRP, E], F32, tag="l2")
                nc.vector.scalar_tensor_tensor(out=l2[:], in0=m1_all[:, gb, :],
                                               scalar=NEG, in1=lg[:],
                                               op0=OP.mult, op1=OP.add)
                m2 = sm.tile([P, GRP, 1], F32)
                nc.vector.tensor_reduce(m2[:], l2[:], axis=AX.X, op=OP.max)
                eq2 = tp.tile([P, GRP, E], F32, tag="eq")
                nc.vector.tensor_tensor(out=eq2[:], in0=l2[:],
                                        in1=m2[:].to_broadcast([P, GRP, E]),
                                        op=OP.is_equal)
                cand2 = tp.tile([P, GRP, E], F32, tag="cand")
                nc.vector.tensor_tensor(out=cand2[:], in0=eq2[:], in1=iota_b,
                                        op=OP.mult)
                i2m = sm.tile([P, GRP, 1], F32)
                nc.vector.tensor_reduce(i2m[:], cand2[:], axis=AX.X, op=OP.min)
                nc.vector.tensor_scalar_add(idx2_sb[:, gb], i2m[:, :, 0], BIG)
                nc.vector.tensor_tensor(out=m2_all[:, gb, :], in0=iota_b,
                                        in1=i2m[:].to_broadcast([P, GRP, E]),
                                        op=OP.is_equal)
                nc.vector.tensor_tensor(out=s_all[:, gb, :], in0=m1_all[:, gb, :],
                                        in1=m2_all[:, gb, :], op=OP.add)
                dsc = sm.tile([P, GRP, 1], F32)
                nc.vector.tensor_tensor(out=dsc[:], in0=m2[:], in1=m1[:],
                                        op=OP.subtract)
                edv = sm.tile([P, GRP, 1], F32)
                nc.scalar.activation(edv[:], dsc[:], ACTF.Exp)
                nc.vector.tensor_scalar_add(edv[:], edv[:], 1.0)
                g1t = sm.tile([P, GRP, 1], F32)
                nc.vector.reciprocal(g1t[:], edv[:])
                nc.vector.tensor_copy(g1_sb[:, gb], g1t[:, :, 0])
                nc.vector.tensor_scalar(out=g2_sb[:, gb], in0=g1t[:, :, 0],
                                        scalar1=-1.0, scalar2=1.0,
                                        op0=OP.mult, op1=OP.add)

            # ---- per-half scan + early scatter (positions are prefix-stable) ----
            x_bf = bigp.tile([P, NT, D], BF16)
            cab_sb = bigp.tile([P, NT, E], F32)
            tmp = bigp.tile([P, NT, E], F32)
            trash_t = consts.tile([P, NT], F32)
            nc.vector.memset(trash_t[:], float(TRASH))
            cnt_row = sm.tile([1, E], F32, tag="cntrow")
            lpos = [None, None]
            loc_i16 = [None, None]
            w_sb = []
            for s_i in range(2):
                lp_t = bigp.tile([P, NT], F32, tag=f"lpos{s_i}")
                lc_t = bigp.tile([P, NT], I16, tag=f"loci{s_i}")
                w_t = bigp.tile([P, NT, E], I16, tag=f"w{s_i}")
                lpos[s_i] = lp_t
                loc_i16[s_i] = lc_t
                w_sb.append(w_t)
            HGRP = (NT // GRP) // 2
            HT = NT // 2                      # tiles per half
            HTOK = TOK // 2                   # pairs per scatter call
            for half in range(2):
                for g in range(half * HGRP, (half + 1) * HGRP):
                    do_group(g)
                hs = slice(half * HT, (half + 1) * HT)
                for q in range(2):
                    qs = slice(half * HT + q * (HT // 2),
                               half * HT + (q + 1) * (HT // 2))
                    nc.vector.tensor_copy(x_bf[:, qs, :], x_all[:, qs, :])
                s_flat = s_all[:, hs, :].rearrange("p t e -> p (t e)")
                cab_ps = pl.tile([P, HT * E], F32, tag="cabps")
                nc.tensor.matmul(cab_ps[:], lhsT=sl[:], rhs=s_flat,
                                 start=True, stop=True)
                nc.scalar.copy(cab_sb[:, hs, :].rearrange("p t e -> p (t e)"),
                               cab_ps[:])
                trow_ps = pl.tile([1, HT * E], F32, tag="trow")
                nc.tensor.matmul(trow_ps[:], lhsT=ones_col[:], rhs=s_flat,
                                 start=True, stop=True)
                trow_sb = sm.tile([1, HT * E], F32, tag="trowsb")
                nc.scalar.copy(trow_sb[:], trow_ps[:])
                t_p = sm.tile([HT, E], F32, tag="tp32")
                nc.sync.dma_start(t_p[:], trow_sb[:])
                toff_ps = pl.tile([HT, E], F32, tag="toffps")
                nc.tensor.matmul(toff_ps[:], lhsT=sl[:HT, :HT], rhs=t_p[:],
                                 start=True, stop=True)
                toff_sb = sm.tile([HT, E], F32, tag="toffsb")
                nc.scalar.copy(toff_sb[:], toff_ps[:])
                toff_row = sm.tile([1, HT * E], F32, tag="toffrow")
                nc.sync.dma_start(toff_row[:], toff_sb[:])
                if half == 1:
                    # carry: add half-0 totals to every tile offset (1-partition op)
                    toff_v = toff_row[:].rearrange("p (t e) -> p t e", e=E)
                    nc.vector.tensor_tensor(
                        out=toff_v, in0=toff_v,
                        in1=cnt_row[:, None, :].to_broadcast([1, HT, E]),
                        op=OP.add)
                toff_bc = bigp.tile([P, HT, E], F32, tag="toffbc")
                nc.gpsimd.partition_broadcast(
                    toff_bc[:].rearrange("p t e -> p (t e)"), toff_row[:])
                nc.vector.tensor_add(cab_sb[:, hs, :], cab_sb[:, hs, :],
                                     toff_bc[:])
                # running per-expert totals (counts row for the collective)
                cnt_ps = pl.tile([1, E], F32, tag="cntps")
                nc.tensor.matmul(cnt_ps[:], lhsT=ones_col[:HT, :], rhs=t_p[:],
                                 start=True, stop=True)
                if half == 0:
                    nc.scalar.copy(cnt_row[:], cnt_ps[:])
                else:
                    nc.vector.tensor_add(cnt_row[:], cnt_row[:], cnt_ps[:])
                # local positions + dispatch locations for this half
                for s_i, mask in ((0, m1_all), (1, m2_all)):
                    nc.vector.tensor_tensor(out=tmp[:, hs, :], in0=mask[:, hs, :],
                                            in1=cab_sb[:, hs, :], op=OP.mult)
                    nc.vector.tensor_reduce(lpos[s_i][:, hs], tmp[:, hs, :],
                                            axis=AX.X, op=OP.add)
                for s_i, idxs in ((0, idx1_sb), (1, idx2_sb)):
                    loc = bigp.tile([P, NT], F32, tag=f"loc{s_i}")
                    nc.vector.scalar_tensor_tensor(out=loc[:, hs], in0=idxs[:, hs],
                                                   scalar=float(MAXC),
                                                   in1=lpos[s_i][:, hs],
                                                   op0=OP.mult, op1=OP.add)
                    over = bigp.tile([P, NT], mybir.dt.uint8, tag=f"over{s_i}")
                    nc.vector.tensor_scalar(out=over[:, hs], in0=lpos[s_i][:, hs],
                                            scalar1=float(MAXC), scalar2=None,
                                            op0=OP.is_ge)
                    nc.vector.select(out=loc[:, hs], mask=over[:, hs],
                                     on_true=trash_t[:, hs], on_false=loc[:, hs])
                    nc.vector.tensor_copy(loc_i16[s_i][:, hs], loc[:, hs])
                # wrapped scatter tiles + the two scatter calls for this half
                for s_i in range(2):
                    wt = w_sb[s_i]
                    for c in range(8):
                        nc.sync.dma_start(wt[0:16, hs, c],
                                          loc_i16[s_i][16 * c:16 * c + 16, hs])
                    for rep in (16, 32, 64):
                        nc.sync.dma_start(wt[rep:2 * rep, hs, :], wt[0:rep, hs, :])
                    nc.gpsimd.dma_scatter_add(
                        xin_bf[:], x_bf[:, hs, :],
                        wt[:, hs, :].rearrange("p t e -> p (t e)"),
                        HTOK, HTOK, D)
            # counts row -> collective input bounce
            cin_b = dram.tile([1, E], F32)
            nc.sync.dma_start(cin_b[:], cnt_row[:])
            # combine-gather wrapped indices (full)
            wg_sb = bigp.tile([P, NT, 16], I16)
            for c in range(16):
                src_l = loc_i16[0] if c < 8 else loc_i16[1]
                cc = c % 8
                nc.sync.dma_start(wg_sb[0:16, :, c], src_l[16 * cc:16 * cc + 16, :])
            for rep in (16, 32, 64):
                nc.sync.dma_start(wg_sb[rep:2 * rep], wg_sb[0:rep])

        # =================== collective ===================
        cout_b = dram.tile([NCORES, E], F32, addr_space="Shared")
        nc.gpsimd.collective_compute(
            "AllGather", OP.bypass,
            ins=[cin_b.opt()], outs=[cout_b.opt()],
            replica_groups=[list(range(NCORES))])
        cnts_sb = consts.tile([NCORES, E], F32)
        nc.sync.dma_start(cnts_sb[:], cout_b[:])

        # zero the trash tile of y_all
        ztile = consts.tile([P, D], BF16)
        nc.vector.memset(ztile[:], 0.0)
        nc.sync.dma_start(y_all[TRASH:TRASH + P, :], ztile[:])

        # =================== expert MLP ===================
        with tc.tile_pool(name="wts", bufs=2) as wts, \
             tc.tile_pool(name="work", bufs=4) as work, \
             tc.tile_pool(name="smp", bufs=6) as smp, \
             tc.tile_pool(name="psB", bufs=1, space="PSUM") as psB:

            ones1 = consts.tile([1, P], BF16)
            nc.vector.memset(ones1[:], 1.0)
            for e in range(E):
                wa = wts.tile([P, 2, D + 1], BF16, tag="wa")
                nc.gpsimd.dma_start(wa[:, :, :D], w1d[e].rearrange("(k p) h -> p k h", p=P))
                wb = wts.tile([P, 2, D + 1], BF16, tag="wb")
                nc.gpsimd.dma_start(wb[:, :, :D], w2d[e].rearrange("(k p) h -> p k h", p=P))
                with nc.allow_low_precision(reason="fp16 row-sum cols"):
                    for k in range(2):
                        nc.vector.tensor_reduce(wa[:, k, D:D + 1], wa[:, k, :D],
                                                axis=AX.X, op=OP.add)
                        nc.vector.tensor_reduce(wb[:, k, D:D + 1], wb[:, k, :D],
                                                axis=AX.X, op=OP.add)
                b1r = wts.tile([1, D + 1], BF16, tag="b1r")
                nc.gpsimd.dma_start(b1r[:, :D], b1d[e][None, :])
                with nc.allow_low_precision(reason="fp16 bias sum col"):
                    nc.vector.tensor_reduce(b1r[:, D:D + 1], b1r[:, :D], axis=AX.X,
                                            op=OP.add)
                b2r = wts.tile([1, D + 1], BF16, tag="b2r")
                nc.gpsimd.dma_start(b2r[:, :D], b2d[e][None, :])
                with nc.allow_low_precision(reason="fp16 bias sum col"):
                    nc.vector.tensor_reduce(b2r[:, D:D + 1], b2r[:, :D], axis=AX.X,
                                            op=OP.add)
                if ln_ident:
                    s1bc = c1bc = s2bc = c2bc = None
                else:
                    s1bc = wts.tile([P, D], BF16, tag="s1bc")
                    nc.gpsimd.dma_start(s1bc[:], s1d[e][None, :].to_broadcast([P, D]))
                    c1bc = wts.tile([P, D], BF16, tag="c1bc")
                    nc.gpsimd.dma_start(c1bc[:], c1d[e][None, :].to_broadcast([P, D]))
                    s2bc = wts.tile([P, D], BF16, tag="s2bc")
                    nc.gpsimd.dma_start(s2bc[:], s2d[e][None, :].to_broadcast([P, D]))
                    c2bc = wts.tile([P, D], BF16, tag="c2bc")
                    nc.gpsimd.dma_start(c2bc[:], c2d[e][None, :].to_broadcast([P, D]))

                def stage1(w):
                    row0 = e * MAXC + w * WV * P
                    xrow = work.tile([P, WV, D], BF16, tag="xrow")
                    nc.scalar.dma_start(
                        xrow[:],
                        xin_bf[row0:row0 + WV * P, :].rearrange("(t p) d -> p t d",
                                                                p=P))
                    xts = work.tile([P, 2, WV * P], BF16, tag="xts")
                    for t in range(WV):
                        for k in range(2):
                            xtp_ps = psB.tile([P, P], BF16, tag=f"xtp{t}")
                            nc.tensor.transpose(xtp_ps[:],
                                                xrow[:, t, k * P:(k + 1) * P],
                                                ident16[:])
                            nc.vector.tensor_copy(xts[:, k, t * P:(t + 1) * P],
                                                  xtp_ps[:])
                    h_wav = _mlp_wave(nc, psB, work, smp, eps_t, ones1,
                                      xts, wa, b1r, s1bc, c1bc, relu=True, pfx="u",
                                      ln_ident=ln_ident)
                    hts = work.tile([P, 2, WV * P], BF16, tag="hts")
                    for t in range(WV):
                        for k in range(2):
                            tp_ps = psB.tile([P, P], BF16, tag=f"htp{t}")
                            nc.tensor.transpose(tp_ps[:], h_wav[:, t, k * P:(k + 1) * P],
                                                ident16[:])
                            if k == 0:
                                nc.vector.tensor_copy(hts[:, k, t * P:(t + 1) * P],
                                                      tp_ps[:])
                            else:
                                nc.scalar.copy(hts[:, k, t * P:(t + 1) * P], tp_ps[:])
                    return hts

                def stage2(w, hts):
                    row0 = e * MAXC + w * WV * P
                    y_wav = _mlp_wave(nc, psB, work, smp, eps_t, ones1,
                                      hts, wb, b2r, s2bc, c2bc, relu=False, pfx="v",
                                      ln_ident=ln_ident)
                    nc.scalar.dma_start(
                        y_all[row0:row0 + WV * P, :].rearrange("(t r) d -> r t d",
                                                               r=P),
                        y_wav[:])

                prev = None
                for w in range(ETILES // WV):
                    hts = stage1(w)
                    if prev is not None:
                        stage2(*prev)
                    prev = (w, hts)
                stage2(*prev)

        # =================== combine ===================
        with tc.tile_pool(name="cwk", bufs=2) as work, \
             tc.tile_pool(name="cps", bufs=1, space="PSUM") as psC:

            base_ps = psC.tile([E, 1], F32, tag="ups0")
            nc.tensor.matmul(base_ps[:], lhsT=cnts_sb[:], rhs=mlt_sb[:],
                             start=True, stop=True)
            capq = consts.tile([E, 1], F32)
            nc.vector.tensor_scalar(out=capq[:], in0=base_ps[:], scalar1=-1.0,
                                    scalar2=float(CAP), op0=OP.mult, op1=OP.add)
            cap_ps = psC.tile([1, E], F32, tag="ups1")
            nc.tensor.transpose(cap_ps[:], capq[:], ident[:E, :E])
            cap_row = consts.tile([1, E], F32)
            nc.scalar.copy(cap_row[:], cap_ps[:])
            cap_bc = consts.tile([P, E], F32)
            nc.gpsimd.partition_broadcast(cap_bc[:], cap_row[:])

            gk16 = []
            for s_i, (idxs, lps, gs) in enumerate(
                    ((idx1_sb, lpos[0], g1_sb), (idx2_sb, lpos[1], g2_sb))):
                msk = work.tile([P, NT, E], F32, tag="msk")
                nc.vector.tensor_tensor(
                    out=msk[:], in0=idxs[:, :, None].to_broadcast([P, NT, E]),
                    in1=iota_f[:, None, :].to_broadcast([P, NT, E]), op=OP.is_equal)
                nc.vector.tensor_tensor(
                    out=msk[:], in0=msk[:],
                    in1=cap_bc[:, None, :].to_broadcast([P, NT, E]), op=OP.mult)
                thr = work.tile([P, NT], F32, tag="thr")
                nc.vector.tensor_reduce(thr[:], msk[:], axis=AX.X, op=OP.add)
                kp = work.tile([P, NT], F32, tag="keep")
                nc.vector.tensor_tensor(out=kp[:], in0=lps[:], in1=thr[:],
                                        op=OP.is_lt)
                gkt = bigp.tile([P, NT], BF16, tag=f"gk16_{s_i}")
                nc.vector.tensor_tensor(out=gkt[:], in0=gs[:], in1=kp[:], op=OP.mult)
                gk16.append(gkt)

            CB = 4
            for tb in range(NT // CB):
                cbs = slice(tb * CB, (tb + 1) * CB)
                yg = work.tile([P, CB, 2, D], BF16, tag="yg")
                nc.gpsimd.dma_gather(yg[:].rearrange("p a b d -> p (a b) d"),
                                     y_all[:], wg_sb[:, cbs, :],
                                     CB * 2 * P, CB * 2 * P, D)
                g0 = work.tile([P, CB, D], BF16, tag="g0t")
                nc.vector.tensor_tensor(
                    out=g0[:], in0=yg[:, :, 0, :],
                    in1=gk16[0][:, cbs, None].to_broadcast([P, CB, D]), op=OP.mult)
                g1 = work.tile([P, CB, D], BF16, tag="g1t")
                nc.vector.tensor_tensor(
                    out=g1[:], in0=yg[:, :, 1, :],
                    in1=gk16[1][:, cbs, None].to_broadcast([P, CB, D]), op=OP.mult)
                acc = work.tile([P, CB, D], BF16, tag="acc")
                nc.vector.tensor_tensor(out=acc[:], in0=g0[:], in1=g1[:], op=OP.add)
                nc.vector.tensor_tensor(out=acc[:], in0=acc[:],
                                        in1=x_bf[:, cbs, :], op=OP.add)
                ot = work.tile([P, CB, D], BF16, tag="ot")
                nc.scalar.activation(ot[:], acc[:], ACTF.Relu)
                nc.sync.dma_start(
                    out_o[tb * CB * P:(tb + 1) * CB * P, :].rearrange(
                        "(t r) d -> r t d", r=P),
                    ot[:])

    nc.compile()
    return nc


def _mlp_wave(nc, psB, work, smp, eps_t, ones1, xts, w_sb, b_row, sbc, cbc, relu,
              pfx, ln_ident):
    out_wav = work.tile([P, WV, D], BF16, tag="hwav" if relu else "ywav")
    ups, mus, rstds = [], [], []
    for t in range(WV):
        u_ps = psB.tile([P, D + 1], F32, tag=f"{pfx}ps{t}")
        nc.tensor.matmul(u_ps[:], lhsT=ones1[:], rhs=b_row[:], start=True, stop=False,
                         skip_group_check=True)
        for k in range(2):
            nc.tensor.matmul(u_ps[:], lhsT=xts[:, k, t * P:(t + 1) * P],
                             rhs=w_sb[:, k, :], start=False, stop=(k == 1),
                             skip_group_check=True)
        ups.append(u_ps)
    sqs = []
    for t in range(WV):
        ssq = smp.tile([P, 1], F32, tag=f"{pfx}ssq{t}")
        usq = work.tile([P, D], BF16, tag="usq")
        nc.scalar.activation(usq[:], ups[t][:, :D], ACTF.Square, accum_out=ssq[:])
        sqs.append(ssq)
    for t in range(WV):
        mu = smp.tile([P, 1], F32, tag=f"{pfx}mu{t}")
        nc.vector.tensor_scalar_mul(mu[:], ups[t][:, D:D + 1], 1.0 / D)
        mu2 = smp.tile([P, 1], F32, tag="mu2")
        nc.vector.tensor_tensor(out=mu2[:], in0=mu[:], in1=mu[:], op=OP.mult)
        var = smp.tile([P, 1], F32, tag="var")
        nc.vector.tensor_scalar(out=var[:], in0=sqs[t][:], scalar1=1.0 / D,
                                scalar2=mu2[:], op0=OP.mult, op1=OP.subtract)
        nc.scalar.activation(var[:], var[:], ACTF.Sqrt, bias=eps_t[:])
        rstd = smp.tile([P, 1], F32, tag=f"{pfx}rstd{t}")
        nc.vector.reciprocal(rstd[:], var[:])
        mus.append(mu)
        rstds.append(rstd)
    for t in range(WV):
        if ln_ident and not relu:
            nc.vector.tensor_scalar(out=out_wav[:, t, :], in0=ups[t][:, :D],
                                    scalar1=mus[t][:], scalar2=rstds[t][:],
                                    op0=OP.subtract, op1=OP.mult)
            continue
        z = work.tile([P, D], BF16, tag="z")
        nc.vector.tensor_scalar(out=z[:], in0=ups[t][:, :D], scalar1=mus[t][:],
                                scalar2=rstds[t][:], op0=OP.subtract, op1=OP.mult)
        if ln_ident:
            nc.scalar.activation(out_wav[:, t, :], z[:], ACTF.Relu)
            continue
        t1 = work.tile([P, D], BF16, tag="t1")
        nc.vector.tensor_tensor(out=t1[:], in0=z[:], in1=sbc[:], op=OP.mult)
        if relu:
            hp = work.tile([P, D], BF16, tag="hp")
            nc.vector.tensor_tensor(out=hp[:], in0=t1[:], in1=cbc[:], op=OP.add)
            nc.scalar.activation(out_wav[:, t, :], hp[:], ACTF.Relu)
        else:
            nc.vector.tensor_tensor(out=out_wav[:, t, :], in0=t1[:], in1=cbc[:],
                                    op=OP.add)
    return out_wav


_CACHE = {}


def _program(ln_ident=True):
    if ln_ident not in _CACHE:
        _CACHE[ln_ident] = build_fused(ln_ident)
    return _CACHE[ln_ident]


def _run(nc, x0, weights, **kw):
    in_maps = []
    for c in range(NCORES):
        m = {
            "x": np.ascontiguousarray(x0[c * TOK:(c + 1) * TOK]),
            "mask_lt": (np.arange(NCORES) < c).astype(np.float32)[:, None],
        }
        m.update(weights)
        in_maps.append(m)
    return run_bass_kernel_spmd(nc, in_maps, core_ids=list(range(NCORES)), **kw)


def kernel(x0, Wr, br, W1, b1, ln1_s, ln1_b, W2, b2, ln2_s, ln2_b,
           _collect_times=None):
    ln_ident = bool(
        np.all(np.asarray(ln1_s) == 1.0) and np.all(np.asarray(ln1_b) == 0.0)
        and np.all(np.asarray(ln2_s) == 1.0) and np.all(np.asarray(ln2_b) == 0.0))
    nc = _program(ln_ident)
    x0 = np.ascontiguousarray(np.asarray(x0, np.float32))
    weights = {
        "wr": np.asarray(Wr, np.float32), "br": np.asarray(br, np.float32),
        "w1d": np.asarray(W1, np.float32), "b1d": np.asarray(b1, np.float32),
        "s1d": np.asarray(ln1_s, np.float32), "c1d": np.asarray(ln1_b, np.float32),
        "w2d": np.asarray(W2, np.float32), "b2d": np.asarray(b2, np.float32),
        "s2d": np.asarray(ln2_s, np.float32), "c2d": np.asarray(ln2_b, np.float32),
    }
    res = _run(nc, x0, weights)
    out = np.concatenate(
        [res.results[c]["out"].astype(np.float32) for c in range(NCORES)], axis=0)
    if _collect_times is not None:
        _collect_times.append((res,))
    return out


# revision 10
# speedup vs baseline: 1.1190x; 1.0280x over previous
"""Trainium2 Bass kernel for nn_MoEResBlock — fused single-launch version.

Per core (8192 tokens): router -> top-2 + gates -> hierarchical cumsum
positions -> SWDGE scatter into per-(core,expert) DRAM regions -> on-device
AllGather of per-core expert counts (overlapped with the expert MLP) ->
dense per-expert MLP (fp16 matmuls, PE transposes, LN via augmented mean
column) -> SWDGE gather-combine with exact global-capacity gates ->
residual + relu -> bf16 out (host upconverts).
"""

import sys

for _p in ("/opt/trn_rl_repo",):
    if _p not in sys.path:
        sys.path.insert(0, _p)

from contextlib import ExitStack

import numpy as np

import concourse.bass as bass
import concourse.mybir as mybir
import concourse.tile as tile
from concourse import bacc
from concourse.bass_utils import run_bass_kernel_spmd
from concourse.masks import make_identity

F32 = mybir.dt.float32
I16 = mybir.dt.int16
I32 = mybir.dt.int32
BF16 = mybir.dt.float16  # half dtype for matmul operands (fp16: 10-bit mantissa)
AX = mybir.AxisListType
OP = mybir.AluOpType
ACTF = mybir.ActivationFunctionType

P = 128
D = 256
E = 8
NCORES = 8
TOK = 65536 // NCORES
NT = TOK // P
GRP = 8
MAXC = 2560
ETILES = MAXC // P
WV = 2
TRASH = E * MAXC
XROWS = TRASH + P
CAP = 16384
BIG = 1000.0
NEG = -1.0e30
LN_EPS = 1e-6


def build_fused(ln_ident=True):
    nc = bacc.Bacc("TRN2", target_bir_lowering=False, debug=False)

    x = nc.dram_tensor("x", [TOK, D], F32, kind="ExternalInput")
    wr = nc.dram_tensor("wr", [D, E], F32, kind="ExternalInput")
    br = nc.dram_tensor("br", [E], F32, kind="ExternalInput")
    w1d = nc.dram_tensor("w1d", [E, D, D], F32, kind="ExternalInput")
    b1d = nc.dram_tensor("b1d", [E, D], F32, kind="ExternalInput")
    s1d = nc.dram_tensor("s1d", [E, D], F32, kind="ExternalInput")
    c1d = nc.dram_tensor("c1d", [E, D], F32, kind="ExternalInput")
    w2d = nc.dram_tensor("w2d", [E, D, D], F32, kind="ExternalInput")
    b2d = nc.dram_tensor("b2d", [E, D], F32, kind="ExternalInput")
    s2d = nc.dram_tensor("s2d", [E, D], F32, kind="ExternalInput")
    c2d = nc.dram_tensor("c2d", [E, D], F32, kind="ExternalInput")
    mask_lt = nc.dram_tensor("mask_lt", [NCORES, 1], F32, kind="ExternalInput")

    out_o = nc.dram_tensor("out", [TOK, D], BF16, kind="ExternalOutput")
    # scatter-add target: ExternalOutput => guaranteed zero-initialized
    xin_bf = nc.dram_tensor("xin", [XROWS, D], BF16, kind="ExternalOutput")
    y_all = nc.dram_tensor("y_all", [XROWS, D], BF16)

    with tile.TileContext(nc) as tc, ExitStack() as ctx:
        consts = ctx.enter_context(tc.tile_pool(name="consts", bufs=1))
        bigp = ctx.enter_context(tc.tile_pool(name="bigp", bufs=1))
        dram = ctx.enter_context(tc.tile_pool(name="dram", bufs=2, space="DRAM"))

        ident = consts.tile([P, P], F32)
        make_identity(nc, ident[:])
        ident16 = consts.tile([P, P], BF16)
        nc.vector.tensor_copy(ident16[:], ident[:])
        # SL[p, i] = 1.0 iff p < i
        sl_ci = consts.tile([P, P], I32)
        nc.gpsimd.iota(sl_ci[:], pattern=[[1, P]], base=0, channel_multiplier=0)
        sl_ri = consts.tile([P, P], I32)
        nc.gpsimd.iota(sl_ri[:], pattern=[[0, P]], base=0, channel_multiplier=1)
        sl_c = consts.tile([P, P], F32)
        nc.vector.tensor_copy(sl_c[:], sl_ci[:])
        sl_r = consts.tile([P, P], F32)
        nc.vector.tensor_copy(sl_r[:], sl_ri[:])
        sl = consts.tile([P, P], F32)
        nc.vector.tensor_tensor(out=sl[:], in0=sl_r[:], in1=sl_c[:], op=OP.is_lt)
        iota_i = consts.tile([P, E], I32)
        nc.gpsimd.iota(iota_i[:], pattern=[[1, E]], base=0, channel_multiplier=0)
        iota_f = consts.tile([P, E], F32)
        nc.vector.tensor_copy(iota_f[:], iota_i[:])
        iota_mb = consts.tile([P, E], F32)
        nc.vector.tensor_scalar_add(iota_mb[:], iota_i[:], -BIG)
        ones_col = consts.tile([P, 1], F32)
        nc.vector.memset(ones_col[:], 1.0)
        eps_t = consts.tile([P, 1], F32)
        nc.vector.memset(eps_t[:], LN_EPS)

        br_row = consts.tile([1, E], F32)
        nc.sync.dma_start(br_row[:], br[None, :])
        br_bc = consts.tile([P, E], F32)
        nc.gpsimd.partition_broadcast(br_bc[:], br_row[:])
        wr_sb = consts.tile([P, 2, E], F32)
        nc.sync.dma_start(wr_sb[:], wr.rearrange("(k p) e -> p k e", p=P))
        mlt_sb = consts.tile([NCORES, 1], F32)
        nc.sync.dma_start(mlt_sb[:], mask_lt[:])

        # ---- bulk x load (single read, reused by router/scatter/combine) ----
        x_all = bigp.tile([P, NT, D], F32)
        nc.sync.dma_start(x_all[:], x.rearrange("(t p) d -> p t d", p=P))

        s_all = bigp.tile([P, NT, E], F32)
        m1_all = bigp.tile([P, NT, E], F32)
        m2_all = bigp.tile([P, NT, E], F32)
        idx1_sb = bigp.tile([P, NT], F32)
        idx2_sb = bigp.tile([P, NT], F32)
        g1_sb = bigp.tile([P, NT], F32)
        g2_sb = bigp.tile([P, NT], F32)

        # =================== router ===================
        with tc.tile_pool(name="rxp", bufs=3) as xp, \
             tc.tile_pool(name="rtp", bufs=3) as tp, \
             tc.tile_pool(name="rsm", bufs=4) as sm, \
             tc.tile_pool(name="rps", bufs=2, space="PSUM") as ps, \
             tc.tile_pool(name="rpl", bufs=1, space="PSUM") as pl:

            def do_group(g):
                lg = tp.tile([P, GRP, E], F32)
                for t in range(GRP):
                    ti = g * GRP + t
                    xts = tp.tile([P, 2, P], F32, tag="xts")
                    for k in range(2):
                        xt_ps = ps.tile([P, P], F32)
                        nc.tensor.transpose(xt_ps[:], x_all[:, ti, k * P:(k + 1) * P],
                                            ident[:])
                        nc.scalar.copy(xts[:, k, :], xt_ps[:])
                    lg_ps = ps.tile([P, E], F32, tag="lgps")
                    for k in range(2):
                        nc.tensor.matmul(lg_ps[:], lhsT=xts[:, k, :],
                                         rhs=wr_sb[:, k, :],
                                         start=(k == 0), stop=(k == 1))
                    nc.vector.tensor_add(lg[:, t, :], lg_ps[:], br_bc[:])

                gb = slice(g * GRP, (g + 1) * GRP)
                iota_b = iota_mb[:, None, :].to_broadcast([P, GRP, E])
                m1 = sm.tile([P, GRP, 1], F32)
                nc.vector.tensor_reduce(m1[:], lg[:], axis=AX.X, op=OP.max)
                eq1 = tp.tile([P, GRP, E], F32, tag="eq")
                nc.vector.tensor_tensor(out=eq1[:], in0=lg[:],
                                        in1=m1[:].to_broadcast([P, GRP, E]),
                                        op=OP.is_equal)
                cand = tp.tile([P, GRP, E], F32, tag="cand")
                nc.vector.tensor_tensor(out=cand[:], in0=eq1[:], in1=iota_b,
                                        op=OP.mult)
                i1m = sm.tile([P, GRP, 1], F32)
                nc.vector.tensor_reduce(i1m[:], cand[:], axis=AX.X, op=OP.min)
                nc.vector.tensor_scalar_add(idx1_sb[:, gb], i1m[:, :, 0], BIG)
                nc.vector.tensor_tensor(out=m1_all[:, gb, :], in0=iota_b,
                                        in1=i1m[:].to_broadcast([P, GRP, E]),
                                        op=OP.is_equal)
                l2 = tp.tile([P, GRP, E], F32, tag="l2")
                nc.vector.scalar_tensor_tensor(out=l2[:], in0=m1_all[:, gb, :],
                                               scalar=NEG, in1=lg[:],
                                               op0=OP.mult, op1=OP.add)
                m2 = sm.tile([P, GRP, 1], F32)
                nc.vector.tensor_reduce(m2[:], l2[:], axis=AX.X, op=OP.max)
                eq2 = tp.tile([P, GRP, E], F32, tag="eq")
                nc.vector.tensor_tensor(out=eq2[:], in0=l2[:],
                                        in1=m2[:].to_broadcast([P, GRP, E]),
                                        op=OP.is_equal)
                cand2 = tp.tile([P, GRP, E], F32, tag="cand")
                nc.vector.tensor_tensor(out=cand2[:], in0=eq2[:], in1=iota_b,
                                        op=OP.mult)
                i2m = sm.tile([P, GRP, 1], F32)
                nc.vector.tensor_reduce(i2m[:], cand2[:], axis=AX.X, op=OP.min)
                nc.vector.tensor_scalar_add(idx2_sb[:, gb], i2m[:, :, 0], BIG)
                nc.vector.tensor_tensor(out=m2_all[:, gb, :], in0=iota_b,
                                        in1=i2m[:].to_broadcast([P, GRP, E]),
                                        op=OP.is_equal)
                nc.vector.tensor_tensor(out=s_all[:, gb, :], in0=m1_all[:, gb, :],
                                        in1=m2_all[:, gb, :], op=OP.add)
                dsc = sm.tile([P, GRP, 1], F32)
                nc.vector.tensor_tensor(out=dsc[:], in0=m2[:], in1=m1[:],
                                        op=OP.subtract)
                edv = sm.tile([P, GRP, 1], F32)
                nc.scalar.activation(edv[:], dsc[:], ACTF.Exp)
                nc.vector.tensor_scalar_add(edv[:], edv[:], 1.0)
                g1t = sm.tile([P, GRP, 1], F32)
                nc.vector.reciprocal(g1t[:], edv[:])
                nc.vector.tensor_copy(g1_sb[:, gb], g1t[:, :, 0])
                nc.vector.tensor_scalar(out=g2_sb[:, gb], in0=g1t[:, :, 0],
                                        scalar1=-1.0, scalar2=1.0,
                                        op0=OP.mult, op1=OP.add)

            # ---- per-half scan + early scatter (positions are prefix-stable) ----
            x_bf = bigp.tile([P, NT, D], BF16)
            cab_sb = bigp.tile([P, NT, E], F32)
            tmp = bigp.tile([P, NT, E], F32)
            trash_t = consts.tile([P, NT], F32)
            nc.vector.memset(trash_t[:], float(TRASH))
            cnt_row = sm.tile([1, E], F32, tag="cntrow")
            lpos = [None, None]
            loc_i16 = [None, None]
            w_sb = []
            for s_i in range(2):
                lp_t = bigp.tile([P, NT], F32, tag=f"lpos{s_i}")
                lc_t = bigp.tile([P, NT], I16, tag=f"loci{s_i}")
                w_t = bigp.tile([P, NT, E], I16, tag=f"w{s_i}")
                lpos[s_i] = lp_t
                loc_i16[s_i] = lc_t
                w_sb.append(w_t)
            NSPLIT = 4
            HGRP = (NT // GRP) // NSPLIT
            HT = NT // NSPLIT                 # tiles per split
            HTOK = TOK // NSPLIT              # pairs per scatter call
            for half in range(NSPLIT):
                for g in range(half * HGRP, (half + 1) * HGRP):
                    do_group(g)
                hs = slice(half * HT, (half + 1) * HT)
                nc.vector.tensor_copy(x_bf[:, hs, :], x_all[:, hs, :])
                s_flat = s_all[:, hs, :].rearrange("p t e -> p (t e)")
                cab_ps = pl.tile([P, HT * E], F32, tag="cabps")
                nc.tensor.matmul(cab_ps[:], lhsT=sl[:], rhs=s_flat,
                                 start=True, stop=True)
                nc.scalar.copy(cab_sb[:, hs, :].rearrange("p t e -> p (t e)"),
                               cab_ps[:])
                trow_ps = pl.tile([1, HT * E], F32, tag="trow")
                nc.tensor.matmul(trow_ps[:], lhsT=ones_col[:], rhs=s_flat,
                                 start=True, stop=True)
                trow_sb = sm.tile([1, HT * E], F32, tag="trowsb")
                nc.scalar.copy(trow_sb[:], trow_ps[:])
                t_p = sm.tile([HT, E], F32, tag="tp32")
                nc.sync.dma_start(t_p[:], trow_sb[:])
                toff_ps = pl.tile([HT, E], F32, tag="toffps")
                nc.tensor.matmul(toff_ps[:], lhsT=sl[:HT, :HT], rhs=t_p[:],
                                 start=True, stop=True)
                toff_sb = sm.tile([HT, E], F32, tag="toffsb")
                nc.scalar.copy(toff_sb[:], toff_ps[:])
                toff_row = sm.tile([1, HT * E], F32, tag="toffrow")
                nc.sync.dma_start(toff_row[:], toff_sb[:])
                if half > 0:
                    # carry: add half-0 totals to every tile offset (1-partition op)
                    toff_v = toff_row[:].rearrange("p (t e) -> p t e", e=E)
                    nc.vector.tensor_tensor(
                        out=toff_v, in0=toff_v,
                        in1=cnt_row[:, None, :].to_broadcast([1, HT, E]),
                        op=OP.add)
                toff_bc = bigp.tile([P, HT, E], F32, tag="toffbc")
                nc.gpsimd.partition_broadcast(
                    toff_bc[:].rearrange("p t e -> p (t e)"), toff_row[:])
                nc.vector.tensor_add(cab_sb[:, hs, :], cab_sb[:, hs, :],
                                     toff_bc[:])
                # running per-expert totals (counts row for the collective)
                cnt_ps = pl.tile([1, E], F32, tag="cntps")
                nc.tensor.matmul(cnt_ps[:], lhsT=ones_col[:HT, :], rhs=t_p[:],
                                 start=True, stop=True)
                if half == 0:
                    nc.scalar.copy(cnt_row[:], cnt_ps[:])
                else:
                    nc.vector.tensor_add(cnt_row[:], cnt_row[:], cnt_ps[:])
                # local positions + dispatch locations for this half
                for s_i, mask in ((0, m1_all), (1, m2_all)):
                    nc.vector.tensor_tensor(out=tmp[:, hs, :], in0=mask[:, hs, :],
                                            in1=cab_sb[:, hs, :], op=OP.mult)
                    nc.vector.tensor_reduce(lpos[s_i][:, hs], tmp[:, hs, :],
                                            axis=AX.X, op=OP.add)
                for s_i, idxs in ((0, idx1_sb), (1, idx2_sb)):
                    loc = bigp.tile([P, NT], F32, tag=f"loc{s_i}")
                    nc.vector.scalar_tensor_tensor(out=loc[:, hs], in0=idxs[:, hs],
                                                   scalar=float(MAXC),
                                                   in1=lpos[s_i][:, hs],
                                                   op0=OP.mult, op1=OP.add)
                    over = bigp.tile([P, NT], mybir.dt.uint8, tag=f"over{s_i}")
                    nc.vector.tensor_scalar(out=over[:, hs], in0=lpos[s_i][:, hs],
                                            scalar1=float(MAXC), scalar2=None,
                                            op0=OP.is_ge)
                    nc.vector.select(out=loc[:, hs], mask=over[:, hs],
                                     on_true=trash_t[:, hs], on_false=loc[:, hs])
                    nc.vector.tensor_copy(loc_i16[s_i][:, hs], loc[:, hs])
                # wrapped scatter tiles + the two scatter calls for this half
                for s_i in range(2):
                    wt = w_sb[s_i]
                    for c in range(8):
                        nc.sync.dma_start(wt[0:16, hs, c],
                                          loc_i16[s_i][16 * c:16 * c + 16, hs])
                    for rep in (16, 32, 64):
                        nc.sync.dma_start(wt[rep:2 * rep, hs, :], wt[0:rep, hs, :])
                    nc.gpsimd.dma_scatter_add(
                        xin_bf[:], x_bf[:, hs, :],
                        wt[:, hs, :].rearrange("p t e -> p (t e)"),
                        HTOK, HTOK, D)
            # counts row -> collective input bounce
            cin_b = dram.tile([1, E], F32)
            nc.sync.dma_start(cin_b[:], cnt_row[:])
            # combine-gather wrapped indices (full)
            wg_sb = bigp.tile([P, NT, 16], I16)
            for c in range(16):
                src_l = loc_i16[0] if c < 8 else loc_i16[1]
                cc = c % 8
                nc.sync.dma_start(wg_sb[0:16, :, c], src_l[16 * cc:16 * cc + 16, :])
            for rep in (16, 32, 64):
                nc.sync.dma_start(wg_sb[rep:2 * rep], wg_sb[0:rep])

        # =================== collective ===================
        cout_b = dram.tile([NCORES, E], F32, addr_space="Shared")
        nc.gpsimd.collective_compute(
            "AllGather", OP.bypass,
            ins=[cin_b.opt()], outs=[cout_b.opt()],
            replica_groups=[list(range(NCORES))])
        cnts_sb = consts.tile([NCORES, E], F32)
        nc.sync.dma_start(cnts_sb[:], cout_b[:])

        # zero the trash tile of y_all
        ztile = consts.tile([P, D], BF16)
        nc.vector.memset(ztile[:], 0.0)
        nc.sync.dma_start(y_all[TRASH:TRASH + P, :], ztile[:])

        # =================== expert MLP ===================
        with tc.tile_pool(name="wts", bufs=2) as wts, \
             tc.tile_pool(name="work", bufs=4) as work, \
             tc.tile_pool(name="smp", bufs=6) as smp, \
             tc.tile_pool(name="psB", bufs=1, space="PSUM") as psB:

            ones1 = consts.tile([1, P], BF16)
            nc.vector.memset(ones1[:], 1.0)
            for e in range(E):
                wa = wts.tile([P, 2, D + 1], BF16, tag="wa")
                nc.gpsimd.dma_start(wa[:, :, :D], w1d[e].rearrange("(k p) h -> p k h", p=P))
                wb = wts.tile([P, 2, D + 1], BF16, tag="wb")
                nc.gpsimd.dma_start(wb[:, :, :D], w2d[e].rearrange("(k p) h -> p k h", p=P))
                with nc.allow_low_precision(reason="fp16 row-sum cols"):
                    for k in range(2):
                        nc.vector.tensor_reduce(wa[:, k, D:D + 1], wa[:, k, :D],
                                                axis=AX.X, op=OP.add)
                        nc.vector.tensor_reduce(wb[:, k, D:D + 1], wb[:, k, :D],
                                                axis=AX.X, op=OP.add)
                b1r = wts.tile([1, D + 1], BF16, tag="b1r")
                nc.gpsimd.dma_start(b1r[:, :D], b1d[e][None, :])
                with nc.allow_low_precision(reason="fp16 bias sum col"):
                    nc.vector.tensor_reduce(b1r[:, D:D + 1], b1r[:, :D], axis=AX.X,
                                            op=OP.add)
                b2r = wts.tile([1, D + 1], BF16, tag="b2r")
                nc.gpsimd.dma_start(b2r[:, :D], b2d[e][None, :])
                with nc.allow_low_precision(reason="fp16 bias sum col"):
                    nc.vector.tensor_reduce(b2r[:, D:D + 1], b2r[:, :D], axis=AX.X,
                                            op=OP.add)
                if ln_ident:
                    s1bc = c1bc = s2bc = c2bc = None
                else:
                    s1bc = wts.tile([P, D], BF16, tag="s1bc")
                    nc.gpsimd.dma_start(s1bc[:], s1d[e][None, :].to_broadcast([P, D]))
                    c1bc = wts.tile([P, D], BF16, tag="c1bc")
                    nc.gpsimd.dma_start(c1bc[:], c1d[e][None, :].to_broadcast([P, D]))
                    s2bc = wts.tile([P, D], BF16, tag="s2bc")
                    nc.gpsimd.dma_start(s2bc[:], s2d[e][None, :].to_broadcast([P, D]))
                    c2bc = wts.tile([P, D], BF16, tag="c2bc")
                    nc.gpsimd.dma_start(c2bc[:], c2d[e][None, :].to_broadcast([P, D]))

                def stage1(w):
                    row0 = e * MAXC + w * WV * P
                    xrow = work.tile([P, WV, D], BF16, tag="xrow")
                    nc.scalar.dma_start(
                        xrow[:],
                        xin_bf[row0:row0 + WV * P, :].rearrange("(t p) d -> p t d",
                                                                p=P))
                    xts = work.tile([P, 2, WV * P], BF16, tag="xts")
                    for t in range(WV):
                        for k in range(2):
                            xtp_ps = psB.tile([P, P], BF16, tag=f"xtp{t}")
                            nc.tensor.transpose(xtp_ps[:],
                                                xrow[:, t, k * P:(k + 1) * P],
                                                ident16[:])
                            nc.vector.tensor_copy(xts[:, k, t * P:(t + 1) * P],
                                                  xtp_ps[:])
                    h_wav = _mlp_wave(nc, psB, work, smp, eps_t, ones1,
                                      xts, wa, b1r, s1bc, c1bc, relu=True, pfx="u",
                                      ln_ident=ln_ident)
                    hts = work.tile([P, 2, WV * P], BF16, tag="hts")
                    for t in range(WV):
                        for k in range(2):
                            tp_ps = psB.tile([P, P], BF16, tag=f"htp{t}")
                            nc.tensor.transpose(tp_ps[:], h_wav[:, t, k * P:(k + 1) * P],
                                                ident16[:])
                            if k == 0:
                                nc.vector.tensor_copy(hts[:, k, t * P:(t + 1) * P],
                                                      tp_ps[:])
                            else:
                                nc.scalar.copy(hts[:, k, t * P:(t + 1) * P], tp_ps[:])
                    return hts

                def stage2(w, hts):
                    row0 = e * MAXC + w * WV * P
                    y_wav = _mlp_wave(nc, psB, work, smp, eps_t, ones1,
                                      hts, wb, b2r, s2bc, c2bc, relu=False, pfx="v",
                                      ln_ident=ln_ident)
                    nc.scalar.dma_start(
                        y_all[row0:row0 + WV * P, :].rearrange("(t r) d -> r t d",
                                                               r=P),
                        y_wav[:])

                prev = None
                for w in range(ETILES // WV):
                    hts = stage1(w)
                    if prev is not None:
                        stage2(*prev)
                    prev = (w, hts)
                stage2(*prev)

        # =================== combine ===================
        with tc.tile_pool(name="cwk", bufs=2) as work, \
             tc.tile_pool(name="cps", bufs=1, space="PSUM") as psC:

            base_ps = psC.tile([E, 1], F32, tag="ups0")
            nc.tensor.matmul(base_ps[:], lhsT=cnts_sb[:], rhs=mlt_sb[:],
                             start=True, stop=True)
            capq = consts.tile([E, 1], F32)
            nc.vector.tensor_scalar(out=capq[:], in0=base_ps[:], scalar1=-1.0,
                                    scalar2=float(CAP), op0=OP.mult, op1=OP.add)
            cap_ps = psC.tile([1, E], F32, tag="ups1")
            nc.tensor.transpose(cap_ps[:], capq[:], ident[:E, :E])
            cap_row = consts.tile([1, E], F32)
            nc.scalar.copy(cap_row[:], cap_ps[:])
            cap_bc = consts.tile([P, E], F32)
            nc.gpsimd.partition_broadcast(cap_bc[:], cap_row[:])

            gk16 = []
            for s_i, (idxs, lps, gs) in enumerate(
                    ((idx1_sb, lpos[0], g1_sb), (idx2_sb, lpos[1], g2_sb))):
                msk = work.tile([P, NT, E], F32, tag="msk")
                nc.vector.tensor_tensor(
                    out=msk[:], in0=idxs[:, :, None].to_broadcast([P, NT, E]),
                    in1=iota_f[:, None, :].to_broadcast([P, NT, E]), op=OP.is_equal)
                nc.vector.tensor_tensor(
                    out=msk[:], in0=msk[:],
                    in1=cap_bc[:, None, :].to_broadcast([P, NT, E]), op=OP.mult)
                thr = work.tile([P, NT], F32, tag="thr")
                nc.vector.tensor_reduce(thr[:], msk[:], axis=AX.X, op=OP.add)
                kp = work.tile([P, NT], F32, tag="keep")
                nc.vector.tensor_tensor(out=kp[:], in0=lps[:], in1=thr[:],
                                        op=OP.is_lt)
                gkt = bigp.tile([P, NT], BF16, tag=f"gk16_{s_i}")
                nc.vector.tensor_tensor(out=gkt[:], in0=gs[:], in1=kp[:], op=OP.mult)
                gk16.append(gkt)

            CB = 4
            for tb in range(NT // CB):
                cbs = slice(tb * CB, (tb + 1) * CB)
                yg = work.tile([P, CB, 2, D], BF16, tag="yg")
                nc.gpsimd.dma_gather(yg[:].rearrange("p a b d -> p (a b) d"),
                                     y_all[:], wg_sb[:, cbs, :],
                                     CB * 2 * P, CB * 2 * P, D)
                g0 = work.tile([P, CB, D], BF16, tag="g0t")
                nc.vector.tensor_tensor(
                    out=g0[:], in0=yg[:, :, 0, :],
                    in1=gk16[0][:, cbs, None].to_broadcast([P, CB, D]), op=OP.mult)
                g1 = work.tile([P, CB, D], BF16, tag="g1t")
                nc.vector.tensor_tensor(
                    out=g1[:], in0=yg[:, :, 1, :],
                    in1=gk16[1][:, cbs, None].to_broadcast([P, CB, D]), op=OP.mult)
                acc = work.tile([P, CB, D], BF16, tag="acc")
                nc.vector.tensor_tensor(out=acc[:], in0=g0[:], in1=g1[:], op=OP.add)
                nc.vector.tensor_tensor(out=acc[:], in0=acc[:],
                                        in1=x_bf[:, cbs, :], op=OP.add)
                ot = work.tile([P, CB, D], BF16, tag="ot")
                nc.scalar.activation(ot[:], acc[:], ACTF.Relu)
                nc.sync.dma_start(
                    out_o[tb * CB * P:(tb + 1) * CB * P, :].rearrange(
                        "(t r) d -> r t d", r=P),
                    ot[:])

    nc.compile()
    return nc


def _mlp_wave(nc, psB, work, smp, eps_t, ones1, xts, w_sb, b_row, sbc, cbc, relu,
              pfx, ln_ident):
    out_wav = work.tile([P, WV, D], BF16, tag="hwav" if relu else "ywav")
    ups, mus, rstds = [], [], []
    for t in range(WV):
        u_ps = psB.tile([P, D + 1], F32, tag=f"{pfx}ps{t}")
        nc.tensor.matmul(u_ps[:], lhsT=ones1[:], rhs=b_row[:], start=True, stop=False,
                         skip_group_check=True)
        for k in range(2):
            nc.tensor.matmul(u_ps[:], lhsT=xts[:, k, t * P:(t + 1) * P],
                             rhs=w_sb[:, k, :], start=False, stop=(k == 1),
                             skip_group_check=True)
        ups.append(u_ps)
    sqs = []
    for t in range(WV):
        ssq = smp.tile([P, 1], F32, tag=f"{pfx}ssq{t}")
        usq = work.tile([P, D], BF16, tag="usq")
        nc.scalar.activation(usq[:], ups[t][:, :D], ACTF.Square, accum_out=ssq[:])
        sqs.append(ssq)
    for t in range(WV):
        mu = smp.tile([P, 1], F32, tag=f"{pfx}mu{t}")
        nc.vector.tensor_scalar_mul(mu[:], ups[t][:, D:D + 1], 1.0 / D)
        mu2 = smp.tile([P, 1], F32, tag="mu2")
        nc.vector.tensor_tensor(out=mu2[:], in0=mu[:], in1=mu[:], op=OP.mult)
        var = smp.tile([P, 1], F32, tag="var")
        nc.vector.tensor_scalar(out=var[:], in0=sqs[t][:], scalar1=1.0 / D,
                                scalar2=mu2[:], op0=OP.mult, op1=OP.subtract)
        nc.scalar.activation(var[:], var[:], ACTF.Sqrt, bias=eps_t[:])
        rstd = smp.tile([P, 1], F32, tag=f"{pfx}rstd{t}")
        nc.vector.reciprocal(rstd[:], var[:])
        mus.append(mu)
        rstds.append(rstd)
    for t in range(WV):
        if ln_ident and not relu:
            nc.vector.tensor_scalar(out=out_wav[:, t, :], in0=ups[t][:, :D],
                                    scalar1=mus[t][:], scalar2=rstds[t][:],
                                    op0=OP.subtract, op1=OP.mult)
            continue
        z = work.tile([P, D], BF16, tag="z")
        nc.vector.tensor_scalar(out=z[:], in0=ups[t][:, :D], scalar1=mus[t][:],
                                scalar2=rstds[t][:], op0=OP.subtract, op1=OP.mult)
        if ln_ident:
            nc.scalar.activation(out_wav[:, t, :], z[:], ACTF.Relu)
            continue
        t1 = work.tile([P, D], BF16, tag="t1")
        nc.vector.tensor_tensor(out=t1[:], in0=z[:], in1=sbc[:], op=OP.mult)
        if relu:
            hp = work.tile([P, D], BF16, tag="hp")
            nc.vector.tensor_tensor(out=hp[:], in0=t1[:], in1=cbc[:], op=OP.add)
            nc.scalar.activation(out_wav[:, t, :], hp[:], ACTF.Relu)
        else:
            nc.vector.tensor_tensor(out=out_wav[:, t, :], in0=t1[:], in1=cbc[:],
                                    op=OP.add)
    return out_wav


_CACHE = {}


def _program(ln_ident=True):
    if ln_ident not in _CACHE:
        _CACHE[ln_ident] = build_fused(ln_ident)
    return _CACHE[ln_ident]


def _run(nc, x0, weights, **kw):
    in_maps = []
    for c in range(NCORES):
        m = {
            "x": np.ascontiguousarray(x0[c * TOK:(c + 1) * TOK]),
            "mask_lt": (np.arange(NCORES) < c).astype(np.float32)[:, None],
        }
        m.update(weights)
        in_maps.append(m)
    return run_bass_kernel_spmd(nc, in_maps, core_ids=list(range(NCORES)), **kw)


def kernel(x0, Wr, br, W1, b1, ln1_s, ln1_b, W2, b2, ln2_s, ln2_b,
           _collect_times=None):
    ln_ident = bool(
        np.all(np.asarray(ln1_s) == 1.0) and np.all(np.asarray(ln1_b) == 0.0)
        and np.all(np.asarray(ln2_s) == 1.0) and np.all(np.asarray(ln2_b) == 0.0))
    nc = _program(ln_ident)
    x0 = np.ascontiguousarray(np.asarray(x0, np.float32))
    weights = {
        "wr": np.asarray(Wr, np.float32), "br": np.asarray(br, np.float32),
        "w1d": np.asarray(W1, np.float32), "b1d": np.asarray(b1, np.float32),
        "s1d": np.asarray(ln1_s, np.float32), "c1d": np.asarray(ln1_b, np.float32),
        "w2d": np.asarray(W2, np.float32), "b2d": np.asarray(b2, np.float32),
        "s2d": np.asarray(ln2_s, np.float32), "c2d": np.asarray(ln2_b, np.float32),
    }
    res = _run(nc, x0, weights)
    out = np.concatenate(
        [res.results[c]["out"].astype(np.float32) for c in range(NCORES)], axis=0)
    if _collect_times is not None:
        _collect_times.append((res,))
    return out


# revision 11
# speedup vs baseline: 1.1474x; 1.0254x over previous
"""Trainium2 Bass kernel for nn_MoEResBlock — fused single-launch version.

Per core (8192 tokens): router -> top-2 + gates -> hierarchical cumsum
positions -> SWDGE scatter into per-(core,expert) DRAM regions -> on-device
AllGather of per-core expert counts (overlapped with the expert MLP) ->
dense per-expert MLP (fp16 matmuls, PE transposes, LN via augmented mean
column) -> SWDGE gather-combine with exact global-capacity gates ->
residual + relu -> bf16 out (host upconverts).
"""

import sys

for _p in ("/opt/trn_rl_repo",):
    if _p not in sys.path:
        sys.path.insert(0, _p)

from contextlib import ExitStack

import numpy as np

import concourse.bass as bass
import concourse.mybir as mybir
import concourse.tile as tile
from concourse import bacc
from concourse.bass_utils import run_bass_kernel_spmd
from concourse.masks import make_identity

F32 = mybir.dt.float32
I16 = mybir.dt.int16
I32 = mybir.dt.int32
BF16 = mybir.dt.float16  # half dtype for matmul operands (fp16: 10-bit mantissa)
AX = mybir.AxisListType
OP = mybir.AluOpType
ACTF = mybir.ActivationFunctionType

P = 128
D = 256
E = 8
NCORES = 8
TOK = 65536 // NCORES
NT = TOK // P
GRP = 8
MAXC = 2560
ETILES = MAXC // P
WV = 2
TRASH = E * MAXC
XROWS = TRASH + P
CAP = 16384
BIG = 1000.0
NEG = -1.0e30
LN_EPS = 1e-6


def build_fused(ln_ident=True):
    nc = bacc.Bacc("TRN2", target_bir_lowering=False, debug=False)

    x = nc.dram_tensor("x", [TOK, D], F32, kind="ExternalInput")
    wr = nc.dram_tensor("wr", [D, E], F32, kind="ExternalInput")
    br = nc.dram_tensor("br", [E], F32, kind="ExternalInput")
    w1d = nc.dram_tensor("w1d", [E, D, D], F32, kind="ExternalInput")
    b1d = nc.dram_tensor("b1d", [E, D], F32, kind="ExternalInput")
    s1d = nc.dram_tensor("s1d", [E, D], F32, kind="ExternalInput")
    c1d = nc.dram_tensor("c1d", [E, D], F32, kind="ExternalInput")
    w2d = nc.dram_tensor("w2d", [E, D, D], F32, kind="ExternalInput")
    b2d = nc.dram_tensor("b2d", [E, D], F32, kind="ExternalInput")
    s2d = nc.dram_tensor("s2d", [E, D], F32, kind="ExternalInput")
    c2d = nc.dram_tensor("c2d", [E, D], F32, kind="ExternalInput")
    mask_lt = nc.dram_tensor("mask_lt", [NCORES, 1], F32, kind="ExternalInput")

    out_o = nc.dram_tensor("out", [TOK, D], BF16, kind="ExternalOutput")
    # scatter-add target: ExternalOutput => guaranteed zero-initialized
    xin_bf = nc.dram_tensor("xin", [XROWS, D], BF16, kind="ExternalOutput")
    y_all = nc.dram_tensor("y_all", [XROWS, D], BF16)

    with tile.TileContext(nc) as tc, ExitStack() as ctx:
        consts = ctx.enter_context(tc.tile_pool(name="consts", bufs=1))
        bigp = ctx.enter_context(tc.tile_pool(name="bigp", bufs=1))
        dram = ctx.enter_context(tc.tile_pool(name="dram", bufs=2, space="DRAM"))

        ident = consts.tile([P, P], F32)
        make_identity(nc, ident[:])
        ident16 = consts.tile([P, P], BF16)
        nc.vector.tensor_copy(ident16[:], ident[:])
        # SL[p, i] = 1.0 iff p < i
        sl_ci = consts.tile([P, P], I32)
        nc.gpsimd.iota(sl_ci[:], pattern=[[1, P]], base=0, channel_multiplier=0)
        sl_ri = consts.tile([P, P], I32)
        nc.gpsimd.iota(sl_ri[:], pattern=[[0, P]], base=0, channel_multiplier=1)
        sl_c = consts.tile([P, P], F32)
        nc.vector.tensor_copy(sl_c[:], sl_ci[:])
        sl_r = consts.tile([P, P], F32)
        nc.vector.tensor_copy(sl_r[:], sl_ri[:])
        sl = consts.tile([P, P], F32)
        nc.vector.tensor_tensor(out=sl[:], in0=sl_r[:], in1=sl_c[:], op=OP.is_lt)
        iota_i = consts.tile([P, E], I32)
        nc.gpsimd.iota(iota_i[:], pattern=[[1, E]], base=0, channel_multiplier=0)
        iota_f = consts.tile([P, E], F32)
        nc.vector.tensor_copy(iota_f[:], iota_i[:])
        iota_mb = consts.tile([P, E], F32)
        nc.vector.tensor_scalar_add(iota_mb[:], iota_i[:], -BIG)
        ones_col = consts.tile([P, 1], F32)
        nc.vector.memset(ones_col[:], 1.0)
        eps_t = consts.tile([P, 1], F32)
        nc.vector.memset(eps_t[:], LN_EPS)

        br_row = consts.tile([1, E], F32)
        nc.sync.dma_start(br_row[:], br[None, :])
        br_bc = consts.tile([P, E], F32)
        nc.gpsimd.partition_broadcast(br_bc[:], br_row[:])
        wr_sb = consts.tile([P, 2, E], F32)
        nc.sync.dma_start(wr_sb[:], wr.rearrange("(k p) e -> p k e", p=P))
        mlt_sb = consts.tile([NCORES, 1], F32)
        nc.sync.dma_start(mlt_sb[:], mask_lt[:])

        # ---- bulk x load (single read, reused by router/scatter/combine) ----
        x_all = bigp.tile([P, NT, D], F32)
        nc.sync.dma_start(x_all[:], x.rearrange("(t p) d -> p t d", p=P))

        s_all = bigp.tile([P, NT, E], F32)
        m1_all = bigp.tile([P, NT, E], F32)
        m2_all = bigp.tile([P, NT, E], F32)
        idx1_sb = bigp.tile([P, NT], F32)
        idx2_sb = bigp.tile([P, NT], F32)
        g1_sb = bigp.tile([P, NT], F32)
        g2_sb = bigp.tile([P, NT], F32)

        # =================== router ===================
        with tc.tile_pool(name="rxp", bufs=3) as xp, \
             tc.tile_pool(name="rtp", bufs=3) as tp, \
             tc.tile_pool(name="rsm", bufs=4) as sm, \
             tc.tile_pool(name="rps", bufs=2, space="PSUM") as ps, \
             tc.tile_pool(name="rpl", bufs=1, space="PSUM") as pl:

            def do_group(g):
                lg = tp.tile([P, GRP, E], F32)
                for t in range(GRP):
                    ti = g * GRP + t
                    xts = tp.tile([P, 2, P], F32, tag="xts")
                    for k in range(2):
                        xt_ps = ps.tile([P, P], F32)
                        nc.tensor.transpose(xt_ps[:], x_all[:, ti, k * P:(k + 1) * P],
                                            ident[:])
                        nc.scalar.copy(xts[:, k, :], xt_ps[:])
                    lg_ps = ps.tile([P, E], F32, tag="lgps")
                    for k in range(2):
                        nc.tensor.matmul(lg_ps[:], lhsT=xts[:, k, :],
                                         rhs=wr_sb[:, k, :],
                                         start=(k == 0), stop=(k == 1))
                    nc.vector.tensor_add(lg[:, t, :], lg_ps[:], br_bc[:])

                gb = slice(g * GRP, (g + 1) * GRP)
                iota_b = iota_mb[:, None, :].to_broadcast([P, GRP, E])
                m1 = sm.tile([P, GRP, 1], F32)
                nc.vector.tensor_reduce(m1[:], lg[:], axis=AX.X, op=OP.max)
                eq1 = tp.tile([P, GRP, E], F32, tag="eq")
                nc.vector.tensor_tensor(out=eq1[:], in0=lg[:],
                                        in1=m1[:].to_broadcast([P, GRP, E]),
                                        op=OP.is_equal)
                cand = tp.tile([P, GRP, E], F32, tag="cand")
                nc.vector.tensor_tensor(out=cand[:], in0=eq1[:], in1=iota_b,
                                        op=OP.mult)
                i1m = sm.tile([P, GRP, 1], F32)
                nc.vector.tensor_reduce(i1m[:], cand[:], axis=AX.X, op=OP.min)
                nc.vector.tensor_scalar_add(idx1_sb[:, gb], i1m[:, :, 0], BIG)
                nc.vector.tensor_tensor(out=m1_all[:, gb, :], in0=iota_b,
                                        in1=i1m[:].to_broadcast([P, GRP, E]),
                                        op=OP.is_equal)
                l2 = tp.tile([P, GRP, E], F32, tag="l2")
                nc.vector.scalar_tensor_tensor(out=l2[:], in0=m1_all[:, gb, :],
                                               scalar=NEG, in1=lg[:],
                                               op0=OP.mult, op1=OP.add)
                m2 = sm.tile([P, GRP, 1], F32)
                nc.vector.tensor_reduce(m2[:], l2[:], axis=AX.X, op=OP.max)
                eq2 = tp.tile([P, GRP, E], F32, tag="eq")
                nc.vector.tensor_tensor(out=eq2[:], in0=l2[:],
                                        in1=m2[:].to_broadcast([P, GRP, E]),
                                        op=OP.is_equal)
                cand2 = tp.tile([P, GRP, E], F32, tag="cand")
                nc.vector.tensor_tensor(out=cand2[:], in0=eq2[:], in1=iota_b,
                                        op=OP.mult)
                i2m = sm.tile([P, GRP, 1], F32)
                nc.vector.tensor_reduce(i2m[:], cand2[:], axis=AX.X, op=OP.min)
                nc.vector.tensor_scalar_add(idx2_sb[:, gb], i2m[:, :, 0], BIG)
                nc.vector.tensor_tensor(out=m2_all[:, gb, :], in0=iota_b,
                                        in1=i2m[:].to_broadcast([P, GRP, E]),
                                        op=OP.is_equal)
                nc.vector.tensor_tensor(out=s_all[:, gb, :], in0=m1_all[:, gb, :],
                                        in1=m2_all[:, gb, :], op=OP.add)
                dsc = sm.tile([P, GRP, 1], F32)
                nc.vector.tensor_tensor(out=dsc[:], in0=m2[:], in1=m1[:],
                                        op=OP.subtract)
                edv = sm.tile([P, GRP, 1], F32)
                nc.scalar.activation(edv[:], dsc[:], ACTF.Exp)
                nc.vector.tensor_scalar_add(edv[:], edv[:], 1.0)
                g1t = sm.tile([P, GRP, 1], F32)
                nc.vector.reciprocal(g1t[:], edv[:])
                nc.vector.tensor_copy(g1_sb[:, gb], g1t[:, :, 0])
                nc.vector.tensor_scalar(out=g2_sb[:, gb], in0=g1t[:, :, 0],
                                        scalar1=-1.0, scalar2=1.0,
                                        op0=OP.mult, op1=OP.add)

            # ---- per-half scan + early scatter (positions are prefix-stable) ----
            x_bf = bigp.tile([P, NT, D], BF16)
            cab_sb = bigp.tile([P, NT, E], F32)
            tmp = bigp.tile([P, NT, E], F32)
            trash_t = consts.tile([P, NT], F32)
            nc.vector.memset(trash_t[:], float(TRASH))
            cnt_row = sm.tile([1, E], F32, tag="cntrow")
            lpos = [None, None]
            loc_i16 = [None, None]
            w_sb = []
            for s_i in range(2):
                lp_t = bigp.tile([P, NT], F32, tag=f"lpos{s_i}")
                lc_t = bigp.tile([P, NT], I16, tag=f"loci{s_i}")
                w_t = bigp.tile([P, NT, E], I16, tag=f"w{s_i}")
                lpos[s_i] = lp_t
                loc_i16[s_i] = lc_t
                w_sb.append(w_t)
            NSPLIT = 4
            HGRP = (NT // GRP) // NSPLIT
            HT = NT // NSPLIT                 # tiles per split
            HTOK = TOK // NSPLIT              # pairs per scatter call
            for half in range(NSPLIT):
                for g in range(half * HGRP, (half + 1) * HGRP):
                    do_group(g)
                hs = slice(half * HT, (half + 1) * HT)
                nc.vector.tensor_copy(x_bf[:, hs, :], x_all[:, hs, :])
                s_flat = s_all[:, hs, :].rearrange("p t e -> p (t e)")
                cab_ps = pl.tile([P, HT * E], F32, tag="cabps")
                nc.tensor.matmul(cab_ps[:], lhsT=sl[:], rhs=s_flat,
                                 start=True, stop=True)
                nc.scalar.copy(cab_sb[:, hs, :].rearrange("p t e -> p (t e)"),
                               cab_ps[:])
                trow_ps = pl.tile([1, HT * E], F32, tag="trow")
                nc.tensor.matmul(trow_ps[:], lhsT=ones_col[:], rhs=s_flat,
                                 start=True, stop=True)
                trow_sb = sm.tile([1, HT * E], F32, tag="trowsb")
                nc.scalar.copy(trow_sb[:], trow_ps[:])
                t_p = sm.tile([HT, E], F32, tag="tp32")
                nc.sync.dma_start(t_p[:], trow_sb[:])
                toff_ps = pl.tile([HT, E], F32, tag="toffps")
                nc.tensor.matmul(toff_ps[:], lhsT=sl[:HT, :HT], rhs=t_p[:],
                                 start=True, stop=True)
                toff_sb = sm.tile([HT, E], F32, tag="toffsb")
                nc.scalar.copy(toff_sb[:], toff_ps[:])
                toff_row = sm.tile([1, HT * E], F32, tag="toffrow")
                nc.sync.dma_start(toff_row[:], toff_sb[:])
                if half > 0:
                    # carry: add half-0 totals to every tile offset (1-partition op)
                    toff_v = toff_row[:].rearrange("p (t e) -> p t e", e=E)
                    nc.vector.tensor_tensor(
                        out=toff_v, in0=toff_v,
                        in1=cnt_row[:, None, :].to_broadcast([1, HT, E]),
                        op=OP.add)
                toff_bc = bigp.tile([P, HT, E], F32, tag="toffbc")
                nc.gpsimd.partition_broadcast(
                    toff_bc[:].rearrange("p t e -> p (t e)"), toff_row[:])
                nc.vector.tensor_add(cab_sb[:, hs, :], cab_sb[:, hs, :],
                                     toff_bc[:])
                # running per-expert totals (counts row for the collective)
                cnt_ps = pl.tile([1, E], F32, tag="cntps")
                nc.tensor.matmul(cnt_ps[:], lhsT=ones_col[:HT, :], rhs=t_p[:],
                                 start=True, stop=True)
                if half == 0:
                    nc.scalar.copy(cnt_row[:], cnt_ps[:])
                else:
                    nc.vector.tensor_add(cnt_row[:], cnt_row[:], cnt_ps[:])
                # local positions + dispatch locations for this half
                for s_i, mask in ((0, m1_all), (1, m2_all)):
                    nc.vector.tensor_tensor(out=tmp[:, hs, :], in0=mask[:, hs, :],
                                            in1=cab_sb[:, hs, :], op=OP.mult)
                    nc.vector.tensor_reduce(lpos[s_i][:, hs], tmp[:, hs, :],
                                            axis=AX.X, op=OP.add)
                for s_i, idxs in ((0, idx1_sb), (1, idx2_sb)):
                    loc = bigp.tile([P, NT], F32, tag=f"loc{s_i}")
                    nc.vector.scalar_tensor_tensor(out=loc[:, hs], in0=idxs[:, hs],
                                                   scalar=float(MAXC),
                                                   in1=lpos[s_i][:, hs],
                                                   op0=OP.mult, op1=OP.add)
                    over = bigp.tile([P, NT], mybir.dt.uint8, tag=f"over{s_i}")
                    nc.vector.tensor_scalar(out=over[:, hs], in0=lpos[s_i][:, hs],
                                            scalar1=float(MAXC), scalar2=None,
                                            op0=OP.is_ge)
                    nc.vector.select(out=loc[:, hs], mask=over[:, hs],
                                     on_true=trash_t[:, hs], on_false=loc[:, hs])
                    nc.vector.tensor_copy(loc_i16[s_i][:, hs], loc[:, hs])
                # wrapped scatter tiles + the two scatter calls for this half
                for s_i in range(2):
                    wt = w_sb[s_i]
                    for c in range(8):
                        nc.sync.dma_start(wt[0:16, hs, c],
                                          loc_i16[s_i][16 * c:16 * c + 16, hs])
                    for rep in (16, 32, 64):
                        nc.sync.dma_start(wt[rep:2 * rep, hs, :], wt[0:rep, hs, :])
                    nc.gpsimd.dma_scatter_add(
                        xin_bf[:], x_bf[:, hs, :],
                        wt[:, hs, :].rearrange("p t e -> p (t e)"),
                        HTOK, HTOK, D)
            # counts row -> collective input bounce
            cin_b = dram.tile([1, E], F32)
            nc.sync.dma_start(cin_b[:], cnt_row[:])
            # combine-gather wrapped indices (full)
            wg_sb = bigp.tile([P, NT, 16], I16)
            for c in range(16):
                src_l = loc_i16[0] if c < 8 else loc_i16[1]
                cc = c % 8
                nc.sync.dma_start(wg_sb[0:16, :, c], src_l[16 * cc:16 * cc + 16, :])
            for rep in (16, 32, 64):
                nc.sync.dma_start(wg_sb[rep:2 * rep], wg_sb[0:rep])

        # =================== collective ===================
        cout_b = dram.tile([NCORES, E], F32, addr_space="Shared")
        nc.gpsimd.collective_compute(
            "AllGather", OP.bypass,
            ins=[cin_b.opt()], outs=[cout_b.opt()],
            replica_groups=[list(range(NCORES))])
        cnts_sb = consts.tile([NCORES, E], F32)
        nc.sync.dma_start(cnts_sb[:], cout_b[:])

        # zero the trash tile of y_all
        ztile = consts.tile([P, D], BF16)
        nc.vector.memset(ztile[:], 0.0)
        nc.sync.dma_start(y_all[TRASH:TRASH + P, :], ztile[:])

        # =================== expert MLP ===================
        with tc.tile_pool(name="wts", bufs=2) as wts, \
             tc.tile_pool(name="work", bufs=4) as work, \
             tc.tile_pool(name="smp", bufs=6) as smp, \
             tc.tile_pool(name="psB", bufs=1, space="PSUM") as psB:

            ones1 = consts.tile([1, P], BF16)
            nc.vector.memset(ones1[:], 1.0)
            for e in range(E):
                wa = wts.tile([P, 2, D + 1], BF16, tag="wa")
                nc.gpsimd.dma_start(wa[:, :, :D], w1d[e].rearrange("(k p) h -> p k h", p=P))
                wb = wts.tile([P, 2, D + 1], BF16, tag="wb")
                nc.gpsimd.dma_start(wb[:, :, :D], w2d[e].rearrange("(k p) h -> p k h", p=P))
                with nc.allow_low_precision(reason="fp16 row-sum cols"):
                    for k in range(2):
                        nc.vector.tensor_reduce(wa[:, k, D:D + 1], wa[:, k, :D],
                                                axis=AX.X, op=OP.add)
                        nc.vector.tensor_reduce(wb[:, k, D:D + 1], wb[:, k, :D],
                                                axis=AX.X, op=OP.add)
                b1r = wts.tile([1, D + 1], BF16, tag="b1r")
                nc.gpsimd.dma_start(b1r[:, :D], b1d[e][None, :])
                with nc.allow_low_precision(reason="fp16 bias sum col"):
                    nc.vector.tensor_reduce(b1r[:, D:D + 1], b1r[:, :D], axis=AX.X,
                                            op=OP.add)
                b2r = wts.tile([1, D + 1], BF16, tag="b2r")
                nc.gpsimd.dma_start(b2r[:, :D], b2d[e][None, :])
                with nc.allow_low_precision(reason="fp16 bias sum col"):
                    nc.vector.tensor_reduce(b2r[:, D:D + 1], b2r[:, :D], axis=AX.X,
                                            op=OP.add)
                if ln_ident:
                    s1bc = c1bc = s2bc = c2bc = None
                else:
                    s1bc = wts.tile([P, D], BF16, tag="s1bc")
                    nc.gpsimd.dma_start(s1bc[:], s1d[e][None, :].to_broadcast([P, D]))
                    c1bc = wts.tile([P, D], BF16, tag="c1bc")
                    nc.gpsimd.dma_start(c1bc[:], c1d[e][None, :].to_broadcast([P, D]))
                    s2bc = wts.tile([P, D], BF16, tag="s2bc")
                    nc.gpsimd.dma_start(s2bc[:], s2d[e][None, :].to_broadcast([P, D]))
                    c2bc = wts.tile([P, D], BF16, tag="c2bc")
                    nc.gpsimd.dma_start(c2bc[:], c2d[e][None, :].to_broadcast([P, D]))

                def stage1(w):
                    row0 = e * MAXC + w * WV * P
                    xrow = work.tile([P, WV, D], BF16, tag="xrow")
                    nc.sync.dma_start(
                        xrow[:],
                        xin_bf[row0:row0 + WV * P, :].rearrange("(t p) d -> p t d",
                                                                p=P))
                    xts = work.tile([P, 2, WV * P], BF16, tag="xts")
                    for t in range(WV):
                        for k in range(2):
                            xtp_ps = psB.tile([P, P], BF16, tag=f"xtp{t}")
                            nc.tensor.transpose(xtp_ps[:],
                                                xrow[:, t, k * P:(k + 1) * P],
                                                ident16[:])
                            nc.vector.tensor_copy(xts[:, k, t * P:(t + 1) * P],
                                                  xtp_ps[:])
                    h_wav = _mlp_wave(nc, psB, work, smp, eps_t, ones1,
                                      xts, wa, b1r, s1bc, c1bc, relu=True, pfx="u",
                                      ln_ident=ln_ident)
                    hts = work.tile([P, 2, WV * P], BF16, tag="hts")
                    for t in range(WV):
                        for k in range(2):
                            tp_ps = psB.tile([P, P], BF16, tag=f"htp{t}")
                            nc.tensor.transpose(tp_ps[:], h_wav[:, t, k * P:(k + 1) * P],
                                                ident16[:])
                            if k == 0:
                                nc.vector.tensor_copy(hts[:, k, t * P:(t + 1) * P],
                                                      tp_ps[:])
                            else:
                                nc.scalar.copy(hts[:, k, t * P:(t + 1) * P], tp_ps[:])
                    return hts

                def stage2(w, hts):
                    row0 = e * MAXC + w * WV * P
                    y_wav = _mlp_wave(nc, psB, work, smp, eps_t, ones1,
                                      hts, wb, b2r, s2bc, c2bc, relu=False, pfx="v",
                                      ln_ident=ln_ident)
                    nc.sync.dma_start(
                        y_all[row0:row0 + WV * P, :].rearrange("(t r) d -> r t d",
                                                               r=P),
                        y_wav[:])

                prev = None
                for w in range(ETILES // WV):
                    hts = stage1(w)
                    if prev is not None:
                        stage2(*prev)
                    prev = (w, hts)
                stage2(*prev)

        # =================== combine ===================
        with tc.tile_pool(name="cwk", bufs=2) as work, \
             tc.tile_pool(name="cps", bufs=1, space="PSUM") as psC:

            base_ps = psC.tile([E, 1], F32, tag="ups0")
            nc.tensor.matmul(base_ps[:], lhsT=cnts_sb[:], rhs=mlt_sb[:],
                             start=True, stop=True)
            capq = consts.tile([E, 1], F32)
            nc.vector.tensor_scalar(out=capq[:], in0=base_ps[:], scalar1=-1.0,
                                    scalar2=float(CAP), op0=OP.mult, op1=OP.add)
            cap_ps = psC.tile([1, E], F32, tag="ups1")
            nc.tensor.transpose(cap_ps[:], capq[:], ident[:E, :E])
            cap_row = consts.tile([1, E], F32)
            nc.scalar.copy(cap_row[:], cap_ps[:])
            cap_bc = consts.tile([P, E], F32)
            nc.gpsimd.partition_broadcast(cap_bc[:], cap_row[:])

            gk16 = []
            for s_i, (idxs, lps, gs) in enumerate(
                    ((idx1_sb, lpos[0], g1_sb), (idx2_sb, lpos[1], g2_sb))):
                msk = work.tile([P, NT, E], F32, tag="msk")
                nc.vector.tensor_tensor(
                    out=msk[:], in0=idxs[:, :, None].to_broadcast([P, NT, E]),
                    in1=iota_f[:, None, :].to_broadcast([P, NT, E]), op=OP.is_equal)
                nc.vector.tensor_tensor(
                    out=msk[:], in0=msk[:],
                    in1=cap_bc[:, None, :].to_broadcast([P, NT, E]), op=OP.mult)
                thr = work.tile([P, NT], F32, tag="thr")
                nc.vector.tensor_reduce(thr[:], msk[:], axis=AX.X, op=OP.add)
                kp = work.tile([P, NT], F32, tag="keep")
                nc.vector.tensor_tensor(out=kp[:], in0=lps[:], in1=thr[:],
                                        op=OP.is_lt)
                gkt = bigp.tile([P, NT], BF16, tag=f"gk16_{s_i}")
                nc.vector.tensor_tensor(out=gkt[:], in0=gs[:], in1=kp[:], op=OP.mult)
                gk16.append(gkt)

            CB = 4
            for tb in range(NT // CB):
                cbs = slice(tb * CB, (tb + 1) * CB)
                yg = work.tile([P, CB, 2, D], BF16, tag="yg")
                nc.gpsimd.dma_gather(yg[:].rearrange("p a b d -> p (a b) d"),
                                     y_all[:], wg_sb[:, cbs, :],
                                     CB * 2 * P, CB * 2 * P, D)
                g0 = work.tile([P, CB, D], BF16, tag="g0t")
                nc.vector.tensor_tensor(
                    out=g0[:], in0=yg[:, :, 0, :],
                    in1=gk16[0][:, cbs, None].to_broadcast([P, CB, D]), op=OP.mult)
                g1 = work.tile([P, CB, D], BF16, tag="g1t")
                nc.vector.tensor_tensor(
                    out=g1[:], in0=yg[:, :, 1, :],
                    in1=gk16[1][:, cbs, None].to_broadcast([P, CB, D]), op=OP.mult)
                acc = work.tile([P, CB, D], BF16, tag="acc")
                nc.vector.tensor_tensor(out=acc[:], in0=g0[:], in1=g1[:], op=OP.add)
                nc.vector.tensor_tensor(out=acc[:], in0=acc[:],
                                        in1=x_bf[:, cbs, :], op=OP.add)
                ot = work.tile([P, CB, D], BF16, tag="ot")
                nc.scalar.activation(ot[:], acc[:], ACTF.Relu)
                nc.sync.dma_start(
                    out_o[tb * CB * P:(tb + 1) * CB * P, :].rearrange(
                        "(t r) d -> r t d", r=P),
                    ot[:])

    nc.compile()
    return nc


def _mlp_wave(nc, psB, work, smp, eps_t, ones1, xts, w_sb, b_row, sbc, cbc, relu,
              pfx, ln_ident):
    out_wav = work.tile([P, WV, D], BF16, tag="hwav" if relu else "ywav")
    ups, mus, rstds = [], [], []
    for t in range(WV):
        u_ps = psB.tile([P, D + 1], F32, tag=f"{pfx}ps{t}")
        nc.tensor.matmul(u_ps[:], lhsT=ones1[:], rhs=b_row[:], start=True, stop=False,
                         skip_group_check=True)
        for k in range(2):
            nc.tensor.matmul(u_ps[:], lhsT=xts[:, k, t * P:(t + 1) * P],
                             rhs=w_sb[:, k, :], start=False, stop=(k == 1),
                             skip_group_check=True)
        ups.append(u_ps)
    sqs = []
    for t in range(WV):
        ssq = smp.tile([P, 1], F32, tag=f"{pfx}ssq{t}")
        usq = work.tile([P, D], BF16, tag="usq")
        nc.scalar.activation(usq[:], ups[t][:, :D], ACTF.Square, accum_out=ssq[:])
        sqs.append(ssq)
    for t in range(WV):
        mu = smp.tile([P, 1], F32, tag=f"{pfx}mu{t}")
        nc.vector.tensor_scalar_mul(mu[:], ups[t][:, D:D + 1], 1.0 / D)
        mu2 = smp.tile([P, 1], F32, tag="mu2")
        nc.vector.tensor_tensor(out=mu2[:], in0=mu[:], in1=mu[:], op=OP.mult)
        var = smp.tile([P, 1], F32, tag="var")
        nc.vector.tensor_scalar(out=var[:], in0=sqs[t][:], scalar1=1.0 / D,
                                scalar2=mu2[:], op0=OP.mult, op1=OP.subtract)
        nc.scalar.activation(var[:], var[:], ACTF.Sqrt, bias=eps_t[:])
        rstd = smp.tile([P, 1], F32, tag=f"{pfx}rstd{t}")
        nc.vector.reciprocal(rstd[:], var[:])
        mus.append(mu)
        rstds.append(rstd)
    for t in range(WV):
        if ln_ident and not relu:
            nc.vector.tensor_scalar(out=out_wav[:, t, :], in0=ups[t][:, :D],
                                    scalar1=mus[t][:], scalar2=rstds[t][:],
                                    op0=OP.subtract, op1=OP.mult)
            continue
        z = work.tile([P, D], BF16, tag="z")
        nc.vector.tensor_scalar(out=z[:], in0=ups[t][:, :D], scalar1=mus[t][:],
                                scalar2=rstds[t][:], op0=OP.subtract, op1=OP.mult)
        if ln_ident:
            nc.scalar.activation(out_wav[:, t, :], z[:], ACTF.Relu)
            continue
        t1 = work.tile([P, D], BF16, tag="t1")
        nc.vector.tensor_tensor(out=t1[:], in0=z[:], in1=sbc[:], op=OP.mult)
        if relu:
            hp = work.tile([P, D], BF16, tag="hp")
            nc.vector.tensor_tensor(out=hp[:], in0=t1[:], in1=cbc[:], op=OP.add)
            nc.scalar.activation(out_wav[:, t, :], hp[:], ACTF.Relu)
        else:
            nc.vector.tensor_tensor(out=out_wav[:, t, :], in0=t1[:], in1=cbc[:],
                                    op=OP.add)
    return out_wav


_CACHE = {}


def _program(ln_ident=True):
    if ln_ident not in _CACHE:
        _CACHE[ln_ident] = build_fused(ln_ident)
    return _CACHE[ln_ident]


def _run(nc, x0, weights, **kw):
    in_maps = []
    for c in range(NCORES):
        m = {
            "x": np.ascontiguousarray(x0[c * TOK:(c + 1) * TOK]),
            "mask_lt": (np.arange(NCORES) < c).astype(np.float32)[:, None],
        }
        m.update(weights)
        in_maps.append(m)
    return run_bass_kernel_spmd(nc, in_maps, core_ids=list(range(NCORES)), **kw)


def kernel(x0, Wr, br, W1, b1, ln1_s, ln1_b, W2, b2, ln2_s, ln2_b,
           _collect_times=None):
    ln_ident = bool(
        np.all(np.asarray(ln1_s) == 1.0) and np.all(np.asarray(ln1_b) == 0.0)
        and np.all(np.asarray(ln2_s) == 1.0) and np.all(np.asarray(ln2_b) == 0.0))
    nc = _program(ln_ident)
    x0 = np.ascontiguousarray(np.asarray(x0, np.float32))
    weights = {
        "wr": np.asarray(Wr, np.float32), "br": np.asarray(br, np.float32),
        "w1d": np.asarray(W1, np.float32), "b1d": np.asarray(b1, np.float32),
        "s1d": np.asarray(ln1_s, np.float32), "c1d": np.asarray(ln1_b, np.float32),
        "w2d": np.asarray(W2, np.float32), "b2d": np.asarray(b2, np.float32),
        "s2d": np.asarray(ln2_s, np.float32), "c2d": np.asarray(ln2_b, np.float32),
    }
    res = _run(nc, x0, weights)
    out = np.concatenate(
        [res.results[c]["out"].astype(np.float32) for c in range(NCORES)], axis=0)
    if _collect_times is not None:
        _collect_times.append((res,))
    return out
